# revision 2
# baseline (speedup 1.0000x reference)
"""Distributed GCN (3x GCNConv + MLP) on 8 Trainium2 NeuronCores, v3.

v3 strategy: eliminate per-edge DMA descriptors. The v2 kernel gathered
z[src] rows via SWDGE dma_gather (~640k DMA descriptors per core per run),
which dominates the profiled execution span. v3 instead:

  - keeps the full dis-scaled z-table in SBUF, FEATURE-major
    ([F, node-pairs, 2] bf16), AllGathered per layer (bf16, tight rows);
  - gathers per-edge rows with nc.gpsimd.ap_gather — a GPSIMD *compute*
    instruction (~18ns/idx, zero DMA descriptors, one instruction per
    ~2-4k edges). Index granularity is a node PAIR (d=2 elements of 4B),
    so a DVE select with a host-shipped parity mask picks the right node;
  - PE-transposes each 128-edge tile back to edge-major and reuses the
    v2 one-hot PSUM aggregation (f32 accumulate);
  - z tables are bf16 (v2 used fp8) -> better accuracy;
  - the node table is processed in 2 half-table passes (50KB/partition
    each) to fit SBUF; per-block aggregation: pass 0 copies PSUM to an
    SBUF stage, pass 1 adds into it.
"""
import sys

for _p in ("/opt/trn_rl_repo",):
    if _p not in sys.path:
        sys.path.insert(0, _p)

import numpy as np
import ml_dtypes

import concourse.bass as bass
import concourse.bacc as bacc
import concourse.tile as tile
import concourse.mybir as mybir
from concourse import bass_utils

BF16 = ml_dtypes.bfloat16
F32 = mybir.dt.float32
BF = mybir.dt.bfloat16
F8 = mybir.dt.float8e4
EPS = 1e-5
NC = 8
P = 128
GIDX = 2048      # max gather indices per ap_gather call


def _to_bf(a):
    return np.ascontiguousarray(np.asarray(a, np.float32)).astype(BF16)


def _rep(v):
    v = np.asarray(v, np.float32).reshape(1, -1)
    return np.ascontiguousarray(np.repeat(v, P, 0))


def preprocess(x, edge_index, ln_g, ln_b, W1, b1, bn1_g, bn1_b, bn1_m, bn1_v,
               W2, b2, bn2_g, bn2_b, bn2_m, bn2_v, W3, b3, bn3_g, bn3_b, bn3_m,
               bn3_v, fc1_W, fc1_b, lnc_g, lnc_b, fc2_W, fc2_b):
    N, D = x.shape
    E = edge_index.shape[1]
    H1, H2, H3 = W1.shape[1], W2.shape[1], W3.shape[1]
    HC, C = fc1_W.shape[1], fc2_W.shape[1]
    assert N % NC == 0, N
    NPC = N // NC
    NBLK = (NPC + P - 1) // P
    NPAD = NBLK * P
    NTAB = NC * NPAD
    assert NTAB % 4 == 0
    NHALF = NTAB // 2           # nodes per half-table
    NPAIR = NHALF // 2          # pair elements per half-table
    assert NPAIR <= 32768
    KD = D // P

    src = np.asarray(edge_index[0], np.int64)
    dst = np.asarray(edge_index[1], np.int64)
    deg = np.bincount(dst, minlength=N).astype(np.float32) + 1.0
    dis = 1.0 / np.sqrt(deg)

    # fold LN gain + BN(eval) into weights (identical to v2)
    k1 = bn1_g / np.sqrt(bn1_v + EPS)
    W1f = (np.asarray(ln_g)[:, None] * np.asarray(W1)) * k1[None, :]
    zb1 = (np.asarray(ln_b) @ np.asarray(W1)) * k1
    b1f = np.asarray(b1) * k1 + (bn1_b - bn1_m * k1)
    w1s = W1f.sum(0)
    k2 = bn2_g / np.sqrt(bn2_v + EPS)
    W2f = np.asarray(W2) * k2[None, :]
    b2f = np.asarray(b2) * k2 + (bn2_b - bn2_m * k2)
    k3 = bn3_g / np.sqrt(bn3_v + EPS)
    W3f = np.asarray(W3) * k3[None, :]
    b3f = np.asarray(b3) * k3 + (bn3_b - bn3_m * k3)

    # edges assigned to dst-owner core; srcpad = linear global padded node id
    core_of = dst // NPC
    dloc = dst - core_of * NPC
    srcpad = (src // NPC) * NPAD + (src % NPC)
    half_of = (srcpad // NHALF).astype(np.int64)

    # per (half, block) tile counts, shared across cores
    counts = np.zeros((NC, 2, NBLK), np.int64)
    per_core = []
    for c in range(NC):
        m = core_of == c
        s = srcpad[m]
        d_ = dloc[m]
        h = half_of[m]
        cell = h * NBLK + (d_ >> 7)
        o = np.argsort(cell, kind="stable")
        s, d_, cell = s[o], d_[o], cell[o]
        counts[c] = np.bincount(cell, minlength=2 * NBLK).reshape(2, NBLK)
        per_core.append((s, d_, cell))

    T = -(-counts.max(0) // P)               # [2, NBLK] tiles per cell
    tile_off = np.concatenate([[0], np.cumsum(T.ravel())]).astype(np.int64)
    ntiles = int(tile_off[-1])

    idx16_list, dstrel_list, bmask_list = [], [], []
    for c in range(NC):
        s, d_, cell = per_core[c]
        start = np.searchsorted(cell, np.arange(2 * NBLK))
        pos = np.arange(len(cell)) - start[cell]
        slot = tile_off[cell] * P + pos
        idx_lin = np.zeros(ntiles * P, np.int32)          # pad -> pair 0
        rel_lin = np.full(ntiles * P, 999.0, np.float32)  # pad -> no match
        par_lin = np.zeros(ntiles * P, np.float32)
        loc = s - (s // NHALF) * NHALF                    # id within half
        idx_lin[slot] = (loc >> 1).astype(np.int32)
        par_lin[slot] = (loc & 1).astype(np.float32)
        rel_lin[slot] = (d_ & 127).astype(np.float32)
        assert idx_lin.min() >= 0 and idx_lin.max() < NPAIR
        idx16 = idx_lin.reshape(ntiles * 8, 16).T.astype(np.int16)
        idx16 = np.tile(idx16, (8, 1))                    # [128, ntiles*8]
        dstrel = rel_lin.reshape(ntiles, P).T             # [128, ntiles]
        bmask = par_lin.reshape(ntiles, P).reshape(ntiles * P)
        bmask = np.tile(bmask.reshape(1, -1), (H1, 1))    # [H1, ntiles*128]
        idx16_list.append(np.ascontiguousarray(idx16))
        dstrel_list.append(np.ascontiguousarray(_to_bf(dstrel)))
        bmask_list.append(
            np.ascontiguousarray(bmask.astype(ml_dtypes.float8_e4m3)))

    # per-half groups of blocks: one ap_gather per group, <= GIDX idxs
    groups = {0: [], 1: []}
    for h in range(2):
        b0 = 0
        off = lambda b: int(tile_off[h * NBLK + b])
        while b0 < NBLK:
            nb = 0
            while (b0 + nb < NBLK
                   and (off(b0 + nb) + T[h, b0 + nb] - off(b0)) * P <= GIDX):
                nb += 1
            nb = max(nb, 1)
            groups[h].append((b0, nb))
            b0 += nb

    # per-core node data: x bf16 node-major + feature-major, dis
    xbf = np.asarray(x, np.float32).astype(BF16)
    xp_list, xt_list, disb_list = [], [], []
    for c in range(NC):
        xp = np.zeros((NPAD, D), BF16)
        xp[:NPC] = xbf[c * NPC:(c + 1) * NPC]
        xpm = xp.reshape(NBLK, P, D).transpose(1, 0, 2).reshape(P, NBLK * D)
        xp_list.append(np.ascontiguousarray(xpm))
        xt = np.zeros((P, KD * NPAD), BF16)
        xf = xp.reshape(NPAD, KD, P)
        xt[:] = np.transpose(xf, (2, 1, 0)).reshape(P, KD * NPAD)
        xt_list.append(np.ascontiguousarray(xt))
        db = np.ones(NPAD, np.float32)
        db[:NPC] = dis[c * NPC:(c + 1) * NPC]
        disb_list.append(np.ascontiguousarray(db.reshape(NBLK, P).T))

    iota = np.tile(np.arange(P, dtype=np.float32), (P, 16))
    ident = np.eye(P, dtype=np.float32)

    consts = dict(
        w1=_to_bf(W1f), w2=_to_bf(W2f), w3=_to_bf(W3f),
        fc1w=_to_bf(np.asarray(fc1_W)), fc2w=_to_bf(np.asarray(fc2_W)),
        w1s=_rep(w1s), zb1=_rep(zb1), b1f=_rep(b1f), b2f=_rep(b2f),
        b3f=_rep(b3f), fc1b=_rep(fc1_b), lncg=_rep(lnc_g), lncb=_rep(lnc_b),
        fc2b=_rep(fc2_b), iota=_to_bf(iota), idn=_to_bf(ident),
    )
    in_maps = []
    for c in range(NC):
        m = dict(consts)
        m.update(xp=xp_list[c], xt=xt_list[c], disb=disb_list[c],
                 idx16=idx16_list[c], dstrel=dstrel_list[c],
                 bmask=bmask_list[c])
        in_maps.append(m)

    cfg = dict(N=N, D=D, E=E, H1=H1, H2=H2, H3=H3, HC=HC, C=C, NPC=NPC,
               NBLK=NBLK, NPAD=NPAD, NTAB=NTAB, NPAIR=NPAIR, ntiles=ntiles,
               T=T.tolist(), tile_off=tile_off.tolist(),
               groups={str(k): v for k, v in groups.items()})
    return cfg, in_maps


def build_nc(cfg):
    skip_gather = int(cfg.get("skip_gather", 0))
    skip_sel = int(cfg.get("skip_sel", 0))
    skip_tiles = int(cfg.get("skip_tiles", 0))
    D, H1, H2, H3 = cfg["D"], cfg["H1"], cfg["H2"], cfg["H3"]
    HC, C = cfg["HC"], cfg["C"]
    NBLK, NPAD, NTAB = cfg["NBLK"], cfg["NPAD"], cfg["NTAB"]
    NPAIR = cfg["NPAIR"]
    ntiles, T, tile_off = cfg["ntiles"], cfg["T"], cfg["tile_off"]
    groups = {int(k): v for k, v in cfg["groups"].items()}
    KD = D // P
    HH = [H1, H2, H3]
    # gather channel counts (pad to multiple of 16)
    CHL = [-(-f // 16) * 16 for f in HH]

    nc = bacc.Bacc("TRN2", target_bir_lowering=False, debug=False,
                   num_devices=NC)
    dt = nc.dram_tensor
    ap_xp = dt("xp", [P, NBLK * D], BF, kind="ExternalInput").ap()
    ap_xt = dt("xt", [P, KD * NPAD], BF, kind="ExternalInput").ap()
    ap_disb = dt("disb", [P, NBLK], F32, kind="ExternalInput").ap()
    ap_idx16 = dt("idx16", [P, ntiles * 8], mybir.dt.int16,
                  kind="ExternalInput").ap()
    ap_dstrel = dt("dstrel", [P, ntiles], BF, kind="ExternalInput").ap()
    ap_bmask = dt("bmask", [H1, ntiles * P], F8, kind="ExternalInput").ap()
    ap_w1 = dt("w1", [D, H1], BF, kind="ExternalInput").ap()
    ap_w2 = dt("w2", [H1, H2], BF, kind="ExternalInput").ap()
    ap_w3 = dt("w3", [H2, H3], BF, kind="ExternalInput").ap()
    ap_fc1w = dt("fc1w", [H3, HC], BF, kind="ExternalInput").ap()
    ap_fc2w = dt("fc2w", [HC, C], BF, kind="ExternalInput").ap()
    reps = {}
    for nm, wd in [("w1s", H1), ("zb1", H1), ("b1f", H1), ("b2f", H2),
                   ("b3f", H3), ("fc1b", HC), ("lncg", HC), ("lncb", HC),
                   ("fc2b", C)]:
        reps[nm] = dt(nm, [P, wd], F32, kind="ExternalInput").ap()
    ap_iota = dt("iota", [P, 16 * P], BF, kind="ExternalInput").ap()
    ap_idn = dt("idn", [P, P], BF, kind="ExternalInput").ap()
    ap_out = dt("out", [NPAD, C], F32, kind="ExternalOutput").ap()

    with tile.TileContext(nc) as tc:
        with (
            tc.tile_pool(name="const", bufs=1) as cp,
            tc.tile_pool(name="stage", bufs=1) as st,
            tc.tile_pool(name="work", bufs=3) as wk,
            tc.tile_pool(name="small", bufs=4) as sm,
            tc.tile_pool(name="psA", bufs=2, space="PSUM") as psA,
            tc.tile_pool(name="psZ", bufs=2, space="PSUM") as psZ,
            tc.tile_pool(name="psT", bufs=3, space="PSUM") as psT,
            tc.tile_pool(name="dram", bufs=1, space="DRAM") as dram,
        ):
            def load_const(ap, shape, dtype):
                t = cp.tile(shape, dtype, tag=f"c{ap.tensor.name}",
                            name=f"c{ap.tensor.name}")
                nc.sync.dma_start(t[:], ap)
                return t

            t_w1 = cp.tile([P, KD * H1], BF, tag="w1")
            nc.sync.dma_start(t_w1[:].rearrange("p (k h) -> p k h", h=H1),
                              ap_w1.rearrange("(k p) h -> p k h", p=P))
            t_w2 = load_const(ap_w2, [H1, H2], BF)
            t_w3 = load_const(ap_w3, [H2, H3], BF)
            t_fc1w = load_const(ap_fc1w, [H3, HC], BF)
            t_fc2w = load_const(ap_fc2w, [HC, C], BF)
            t_rep = {}
            for nm in reps:
                t_rep[nm] = load_const(reps[nm], list(reps[nm].shape), F32)
            t_iota = load_const(ap_iota, [P, 16 * P], BF)
            t_idn = load_const(ap_idn, [P, P], BF)
            t_disb = load_const(ap_disb, [P, NBLK], F32)
            t_eps = cp.tile([P, 1], F32, tag="eps")
            nc.vector.memset(t_eps[:], float(EPS))
            t_idx = cp.tile([P, ntiles * 8], mybir.dt.int16, tag="idx")
            nc.sync.dma_start(t_idx[:], ap_idx16)
            t_drel = cp.tile([P, ntiles], BF, tag="drel")
            nc.sync.dma_start(t_drel[:], ap_dstrel)

            # DRAM z tables: local shard (feature-major) + allgathered full
            z_local = [dram.tile([HH[l], NPAD], BF, tag=f"zloc{l}",
                                 name=f"zloc{l}") for l in range(3)]
            z_full = [dram.tile([NC * HH[l], NPAD], BF, tag=f"zfull{l}",
                                name=f"zfull{l}", addr_space="Shared")
                      for l in range(3)]

            zs_buf = [st.tile([P, NBLK * HH[l]], BF, tag=f"zs{l}",
                              name=f"zs{l}") for l in range(3)]
            zs_v = [zs_buf[l][:].rearrange("p (j h) -> p j h", h=HH[l])
                    for l in range(3)]
            t_zT = st.tile([H1, NPAD], BF, tag="zT", name="zT")
            out_buf = st.tile([P, NBLK * C], F32, tag="outb")
            t_hst = st.tile([P, NBLK * H1], F32, tag="hst", name="hst")
            t_hb = st.tile([P, NBLK * H1], BF, tag="hb", name="hb")
            t_z4 = st.tile([P, NBLK * HC], F32, tag="z4", name="z4")
            t_r4 = st.tile([P, NBLK * HC], BF, tag="r4", name="r4")

            def bcast_node(t, w):
                a = t[:]
                return bass.AP(a.tensor, a.offset, a.ap + [[0, w]])

            def bcast_feat(t, w):
                a = t[:]
                return bass.AP(a.tensor, a.offset,
                               [a.ap[0], [0, NBLK], a.ap[1]])

            def transpose_to(dst_sb, src_ap, fh):
                """PE-transpose src [128, fh] -> psum [fh, 128] -> dst sbuf."""
                tp = psT.tile([P, P], BF, tag="tps")
                nc.tensor.transpose(tp[0:fh, :], src_ap, t_idn[:])
                nc.vector.tensor_copy(dst_sb, tp[0:fh, :])

            # ============ phase A: stats + z1 = LN(x) @ W1f (folded) ========
            with tc.tile_pool(name="xin", bufs=1) as xin:
                t_xp = xin.tile([P, NBLK * D], BF, tag="xp")
                nc.sync.dma_start(t_xp[:], ap_xp)
                t_xt = xin.tile([P, KD * NPAD], BF, tag="xt")
                nc.sync.dma_start(t_xt[:], ap_xt)
                t1 = xin.tile([P, NBLK * H1], BF, tag="t1", name="t1")
                t2 = xin.tile([P, NBLK * H1], BF, tag="t2", name="t2")
                ystage = xin.tile([P, NBLK * H1], BF, tag="ystage")

                ssum = sm.tile([P, NBLK], F32, tag="ssum")
                nc.vector.reduce_sum(
                    ssum[:].rearrange("p (j o) -> p j o", o=1),
                    t_xp[:].rearrange("p (j d) -> p j d", d=D),
                    axis=mybir.AxisListType.X)
                s2 = sm.tile([P, NBLK], F32, tag="s2")
                sqscr = wk.tile([P, D], F32, tag="sqscr")
                for b in range(NBLK):
                    nc.scalar.activation(
                        sqscr[:], t_xp[:, b * D:(b + 1) * D],
                        mybir.ActivationFunctionType.Square,
                        accum_out=s2[:, b:b + 1])
                mu = sm.tile([P, NBLK], F32, tag="mu")
                nc.vector.tensor_scalar_mul(mu[:], ssum[:], 1.0 / D)
                musq = sm.tile([P, NBLK], F32, tag="musq")
                nc.vector.tensor_tensor(musq[:], mu[:], mu[:],
                                        op=mybir.AluOpType.mult)
                var = sm.tile([P, NBLK], F32, tag="var")
                nc.vector.tensor_scalar_mul(var[:], s2[:], 1.0 / D)
                nc.vector.tensor_tensor(var[:], var[:], musq[:],
                                        op=mybir.AluOpType.subtract)
                std = sm.tile([P, NBLK], F32, tag="std")
                nc.scalar.activation(std[:], var[:],
                                     mybir.ActivationFunctionType.Sqrt,
                                     bias=t_eps[:], scale=1.0)
                rstd = sm.tile([P, NBLK], F32, tag="rstd")
                nc.vector.reciprocal(rstd[:], std[:])
                a_sc = sm.tile([P, NBLK], F32, tag="a_sc")
                nc.vector.tensor_tensor(a_sc[:], t_disb[:], rstd[:],
                                        op=mybir.AluOpType.mult)
                m2 = sm.tile([P, NBLK], F32, tag="m2")
                nc.vector.tensor_tensor(m2[:], a_sc[:], mu[:],
                                        op=mybir.AluOpType.mult)
                nc.vector.tensor_scalar_mul(m2[:], m2[:], -1.0)

                for b in range(NBLK):
                    zp = psZ.tile([P, H1], F32, tag="zps")
                    for kc in range(KD):
                        nc.tensor.matmul(
                            zp[:],
                            lhsT=t_xt[:, kc * NPAD + b * P:
                                      kc * NPAD + (b + 1) * P],
                            rhs=t_w1[:, kc * H1:(kc + 1) * H1],
                            start=(kc == 0), stop=(kc == KD - 1))
                    nc.vector.tensor_copy(ystage[:, b * H1:(b + 1) * H1],
                                          zp[:])
                nc.vector.tensor_tensor(
                    t1[:].rearrange("p (j h) -> p j h", h=H1),
                    bcast_node(m2, H1), bcast_feat(t_rep["w1s"], H1),
                    op=mybir.AluOpType.mult)
                nc.vector.tensor_tensor(
                    t2[:].rearrange("p (j h) -> p j h", h=H1),
                    bcast_node(t_disb, H1), bcast_feat(t_rep["zb1"], H1),
                    op=mybir.AluOpType.mult)
                nc.vector.tensor_tensor(t1[:], t1[:], t2[:],
                                        op=mybir.AluOpType.add)
                nc.vector.tensor_tensor(
                    t2[:].rearrange("p (j h) -> p j h", h=H1),
                    ystage[:].rearrange("p (j h) -> p j h", h=H1),
                    bcast_node(a_sc, H1), op=mybir.AluOpType.mult)
                nc.vector.tensor_tensor(
                    zs_v[0],
                    t2[:].rearrange("p (j h) -> p j h", h=H1),
                    t1[:].rearrange("p (j h) -> p j h", h=H1),
                    op=mybir.AluOpType.add)

            # z1T = transpose(zs0) feature-major, write local shard
            def write_zT(l):
                fh = HH[l]
                for b in range(NBLK):
                    transpose_to(t_zT[0:fh, b * P:(b + 1) * P],
                                 zs_buf[l][:, b * fh:(b + 1) * fh], fh)
                nc.sync.dma_start(z_local[l][:], t_zT[0:fh, 0:NPAD])

            write_zT(0)

            with (
                tc.tile_pool(name="tabp", bufs=1) as tb,
                tc.tile_pool(name="gath", bufs=2) as gp,
                tc.tile_pool(name="msk", bufs=2) as mp,
                tc.tile_pool(name="onehot", bufs=1) as op_,
                tc.tile_pool(name="ztile", bufs=4) as zp_,
            ):
                # gather table (half): [CH, NPAIR, 2] bf16, NTAB B/partition
                t_tab = tb.tile([P, NTAB // 2], BF, tag="tab", name="tab")
                def edge_layer(l, t_wnext, postbias):
                    Fh = HH[l]
                    ch = CHL[l]
                    nc.gpsimd.collective_compute(
                        "AllGather", mybir.AluOpType.bypass,
                        replica_groups=[list(range(NC))],
                        ins=[z_local[l][:].opt()],
                        outs=[z_full[l][:].opt()],
                    )
                    hstage = t_hst[:, 0:NBLK * Fh]
                    if ch > Fh:
                        # zero the pad feature rows; table loads overwrite
                        # [0:Fh] afterwards (DVE needs 32-aligned partitions)
                        nc.vector.memset(t_tab[0:ch, :], 0.0)
                    for h in range(2):
                        # load half-table: 4 core shards, feature-major
                        for i in range(4):
                            c = 4 * h + i
                            nc.sync.dma_start(
                                t_tab[0:Fh, i * NPAD:(i + 1) * NPAD],
                                z_full[l][c * Fh:(c + 1) * Fh, :])
                        for (b0, nb) in groups[h]:
                            t0 = tile_off[h * NBLK + b0]
                            t1_ = tile_off[h * NBLK + b0 + nb - 1] \
                                + T[h][b0 + nb - 1]
                            gt = t1_ - t0
                            if gt == 0:
                                continue
                            g = gp.tile([ch, GIDX * 2], BF, tag="gbuf")
                            if skip_gather:
                                nc.vector.memset(g[:, 0:1], 0.0)
                            else:
                                nc.gpsimd.ap_gather(
                                    out_ap=g[:, 0:gt * P * 2].rearrange(
                                        "p (n d) -> p n d", d=2),
                                    in_ap=t_tab[0:ch, :].rearrange(
                                        "p (n d) -> p n d", d=2),
                                    idxs_ap=t_idx[0:ch, t0 * 8:t1_ * 8],
                                    channels=ch,
                                    num_elems=NPAIR,
                                    d=2,
                                    num_idxs=gt * P,
                                )
                            mk = mp.tile([Fh, GIDX], F8, tag="mk")
                            nc.sync.dma_start(
                                mk[:, 0:gt * P],
                                ap_bmask[0:Fh, t0 * P:t1_ * P])
                            ze = gp.tile([Fh, GIDX], BF, tag="ze")
                            gv = g[0:Fh, 0:gt * P * 2].rearrange(
                                "p (n d) -> p n d", d=2)
                            # ze = g0 + (g1 - g0) * parity
                            if skip_sel:
                                nc.vector.memset(ze[:, 0:1], 0.0)
                            else:
                                nc.vector.tensor_tensor(
                                    ze[:, 0:gt * P], gv[:, :, 1], gv[:, :, 0],
                                    op=mybir.AluOpType.subtract)
                                nc.vector.tensor_tensor(
                                    ze[:, 0:gt * P], ze[:, 0:gt * P],
                                    mk[:, 0:gt * P], op=mybir.AluOpType.mult)
                                nc.vector.tensor_tensor(
                                    ze[:, 0:gt * P], ze[:, 0:gt * P],
                                    gv[:, :, 0], op=mybir.AluOpType.add)
                            # one-hot tiles for this group
                            oh = op_.tile([P, gt * P], BF, tag="ohS")
                            for s0 in range(0, gt, 16):
                                s1 = min(s0 + 16, gt)
                                dr = t_drel[:, t0 + s0:t0 + s1]
                                dr_b = bass.AP(dr.tensor, dr.offset,
                                               dr.ap + [[0, P]])
                                nc.vector.tensor_tensor(
                                    out=oh[:, s0 * P:s1 * P].rearrange(
                                        "p (t w) -> p t w", w=P),
                                    in0=t_iota[:, 0:(s1 - s0) * P].rearrange(
                                        "p (t w) -> p t w", w=P),
                                    in1=dr_b,
                                    op=mybir.AluOpType.is_equal)
                            for b in range(b0, b0 + nb):
                                nt = T[h][b]
                                if nt == 0:
                                    if h == 0:
                                        nc.vector.memset(
                                            hstage[:, b * Fh:(b + 1) * Fh],
                                            0.0)
                                    continue
                                base = tile_off[h * NBLK + b]
                                agg = psA.tile([P, Fh], F32, tag="agg")
                                if skip_tiles:
                                    nc.vector.memset(agg[:], 0.0)
                                for t in range(nt if not skip_tiles else 0):
                                    gidx = base + t - t0
                                    # ze tile [Fh, 128] -> edge-major [128, Fh]
                                    zt = zp_.tile([P, H1], BF, tag="zt")
                                    tp = psT.tile([P, P], BF, tag="tps")
                                    nc.tensor.transpose(
                                        tp[:, 0:Fh],
                                        ze[:, gidx * P:(gidx + 1) * P],
                                        t_idn[0:Fh, 0:Fh])
                                    nc.vector.tensor_copy(zt[:, 0:Fh],
                                                          tp[:, 0:Fh])
                                    nc.tensor.matmul(
                                        agg[:],
                                        lhsT=oh[:, gidx * P:(gidx + 1) * P],
                                        rhs=zt[:, 0:Fh],
                                        start=(t == 0), stop=(t == nt - 1))
                                dst_sl = hstage[:, b * Fh:(b + 1) * Fh]
                                if h == 0:
                                    nc.vector.tensor_copy(dst_sl, agg[:])
                                else:
                                    nc.vector.tensor_tensor(
                                        dst_sl, dst_sl, agg[:],
                                        op=mybir.AluOpType.add)
                    # epilogue: h = relu(disb*(agg + zs) + bias)
                    nc.vector.tensor_tensor(
                        hstage[:].rearrange("p (j h) -> p j h", h=Fh),
                        hstage[:].rearrange("p (j h) -> p j h", h=Fh),
                        zs_v[l], op=mybir.AluOpType.add)
                    nc.vector.tensor_tensor(
                        hstage[:].rearrange("p (j h) -> p j h", h=Fh),
                        hstage[:].rearrange("p (j h) -> p j h", h=Fh),
                        bcast_node(t_disb, Fh), op=mybir.AluOpType.mult)
                    nc.vector.tensor_tensor(
                        hstage[:].rearrange("p (j h) -> p j h", h=Fh),
                        hstage[:].rearrange("p (j h) -> p j h", h=Fh),
                        bcast_feat(postbias, Fh), op=mybir.AluOpType.add)
                    hb = t_hb[:, 0:NBLK * Fh]
                    nc.scalar.activation(hb[:], hstage[:],
                                         mybir.ActivationFunctionType.Relu)
                    if t_wnext is None:
                        return hb
                    # hstage (t_hst) is dead after relu -> reuse it for znext
                    Fo = HH[l + 1]
                    znext = t_hst[:, 0:NBLK * Fo]
                    for b in range(NBLK):
                        hT = wk.tile([P, P], BF, tag="hT")
                        transpose_to(hT[0:Fh, :], hb[:, b * Fh:(b + 1) * Fh],
                                     Fh)
                        zp2 = psZ.tile([P, Fo], F32, tag="zps")
                        nc.tensor.matmul(zp2[:], lhsT=hT[0:Fh, :],
                                         rhs=t_wnext[:], start=True,
                                         stop=True)
                        nc.vector.tensor_copy(znext[:, b * Fo:(b + 1) * Fo],
                                              zp2[:])
                    nc.vector.tensor_tensor(
                        zs_v[l + 1],
                        znext[:].rearrange("p (j h) -> p j h", h=Fo),
                        bcast_node(t_disb, Fo), op=mybir.AluOpType.mult)
                    write_zT(l + 1)
                    return None

                def classifier(h3):
                    z4 = t_z4
                    for b in range(NBLK):
                        hT = wk.tile([P, P], BF, tag="hT")
                        transpose_to(hT[0:H3, :], h3[:, b * H3:(b + 1) * H3],
                                     H3)
                        zp2 = psZ.tile([P, HC], F32, tag="zps")
                        nc.tensor.matmul(zp2[:], lhsT=hT[0:H3, :],
                                         rhs=t_fc1w[:], start=True, stop=True)
                        nc.vector.tensor_copy(z4[:, b * HC:(b + 1) * HC],
                                              zp2[:])
                    nc.vector.tensor_tensor(
                        z4[:].rearrange("p (j h) -> p j h", h=HC),
                        z4[:].rearrange("p (j h) -> p j h", h=HC),
                        bcast_feat(t_rep["fc1b"], HC), op=mybir.AluOpType.add)
                    ssum = sm.tile([P, NBLK], F32, tag="ssum4")
                    nc.vector.reduce_sum(
                        ssum[:].rearrange("p (j o) -> p j o", o=1),
                        z4[:].rearrange("p (j h) -> p j h", h=HC),
                        axis=mybir.AxisListType.X)
                    mu = sm.tile([P, NBLK], F32, tag="mu4")
                    nc.vector.tensor_scalar_mul(mu[:], ssum[:], 1.0 / HC)
                    zc = wk.tile([P, NBLK * HC], F32, tag="zc")
                    nc.vector.tensor_tensor(
                        zc[:].rearrange("p (j h) -> p j h", h=HC),
                        z4[:].rearrange("p (j h) -> p j h", h=HC),
                        bcast_node(mu, HC), op=mybir.AluOpType.subtract)
                    zsq = wk.tile([P, NBLK * HC], F32, tag="zsq")
                    nc.vector.tensor_tensor(zsq[:], zc[:], zc[:],
                                            op=mybir.AluOpType.mult)
                    var = sm.tile([P, NBLK], F32, tag="var4")
                    nc.vector.reduce_sum(
                        var[:].rearrange("p (j o) -> p j o", o=1),
                        zsq[:].rearrange("p (j h) -> p j h", h=HC),
                        axis=mybir.AxisListType.X)
                    nc.vector.tensor_scalar_mul(var[:], var[:], 1.0 / HC)
                    std = sm.tile([P, NBLK], F32, tag="std4")
                    nc.scalar.activation(std[:], var[:],
                                         mybir.ActivationFunctionType.Sqrt,
                                         bias=t_eps[:], scale=1.0)
                    rstd = sm.tile([P, NBLK], F32, tag="rstd4")
                    nc.vector.reciprocal(rstd[:], std[:])
                    nc.vector.tensor_tensor(
                        zc[:].rearrange("p (j h) -> p j h", h=HC),
                        zc[:].rearrange("p (j h) -> p j h", h=HC),
                        bcast_node(rstd, HC), op=mybir.AluOpType.mult)
                    nc.vector.tensor_tensor(
                        zc[:].rearrange("p (j h) -> p j h", h=HC),
                        zc[:].rearrange("p (j h) -> p j h", h=HC),
                        bcast_feat(t_rep["lncg"], HC), op=mybir.AluOpType.mult)
                    nc.vector.tensor_tensor(
                        zc[:].rearrange("p (j h) -> p j h", h=HC),
                        zc[:].rearrange("p (j h) -> p j h", h=HC),
                        bcast_feat(t_rep["lncb"], HC), op=mybir.AluOpType.add)
                    r4 = t_r4
                    nc.scalar.activation(r4[:], zc[:],
                                         mybir.ActivationFunctionType.Relu)
                    for b in range(NBLK):
                        rT = wk.tile([P, P], BF, tag="rT")
                        transpose_to(rT[0:HC, :], r4[:, b * HC:(b + 1) * HC],
                                     HC)
                        op2 = psZ.tile([P, C], F32, tag="zps")
                        nc.tensor.matmul(op2[:], lhsT=rT[0:HC, :],
                                         rhs=t_fc2w[:], start=True, stop=True)
                        nc.vector.tensor_copy(out_buf[:, b * C:(b + 1) * C],
                                              op2[:])
                    nc.vector.tensor_tensor(
                        out_buf[:].rearrange("p (j c) -> p j c", c=C),
                        out_buf[:].rearrange("p (j c) -> p j c", c=C),
                        bcast_feat(t_rep["fc2b"], C), op=mybir.AluOpType.add)

                edge_layer(0, t_w2, t_rep["b1f"])
                edge_layer(1, t_w3, t_rep["b2f"])
                h3 = edge_layer(2, None, t_rep["b3f"])
                classifier(h3)

            nc.sync.dma_start(
                ap_out.rearrange("(j p) c -> p j c", p=P),
                out_buf[:].rearrange("p (j c) -> p j c", c=C))
    nc.compile()
    return nc


_CACHE = {}


def _get_nc(cfg):
    key = repr(sorted((k, str(v)) for k, v in cfg.items()))
    if key not in _CACHE:
        _CACHE[key] = build_nc(cfg)
    return _CACHE[key]


def kernel(**inputs):
    cfg, in_maps = preprocess(**inputs)
    nc = _get_nc(cfg)
    res = bass_utils.run_bass_kernel_spmd(nc, in_maps, core_ids=list(range(NC)))
    NPC, N, C = cfg["NPC"], cfg["N"], cfg["C"]
    out = np.empty((N, C), np.float32)
    for c in range(NC):
        out[c * NPC:(c + 1) * NPC] = res.results[c]["out"][:NPC]
    return out


# revision 3
# speedup vs baseline: 1.2519x; 1.2519x over previous
"""Distributed GCN (3x GCNConv + MLP) on 8 Trainium2 NeuronCores, v3.

v3 strategy: eliminate per-edge DMA descriptors. The v2 kernel gathered
z[src] rows via SWDGE dma_gather (~640k DMA descriptors per core per run),
which dominates the profiled execution span. v3 instead:

  - keeps the full dis-scaled z-table in SBUF, FEATURE-major
    ([F, node-pairs, 2] bf16), AllGathered per layer (bf16, tight rows);
  - gathers per-edge rows with nc.gpsimd.ap_gather — a GPSIMD *compute*
    instruction (~18ns/idx, zero DMA descriptors, one instruction per
    ~2-4k edges). Index granularity is a node PAIR (d=2 elements of 4B),
    so a DVE select with a host-shipped parity mask picks the right node;
  - PE-transposes each 128-edge tile back to edge-major and reuses the
    v2 one-hot PSUM aggregation (f32 accumulate);
  - z tables are bf16 (v2 used fp8) -> better accuracy;
  - the node table is processed in 2 half-table passes (50KB/partition
    each) to fit SBUF; per-block aggregation: pass 0 copies PSUM to an
    SBUF stage, pass 1 adds into it.
"""
import sys

for _p in ("/opt/trn_rl_repo",):
    if _p not in sys.path:
        sys.path.insert(0, _p)

import numpy as np
import ml_dtypes

import concourse.bass as bass
import concourse.bacc as bacc
import concourse.tile as tile
import concourse.mybir as mybir
from concourse import bass_utils

BF16 = ml_dtypes.bfloat16
F32 = mybir.dt.float32
BF = mybir.dt.bfloat16
F8 = mybir.dt.float8e4
EPS = 1e-5
NC = 8
P = 128
GIDX = 2048      # max gather indices per ap_gather call
MIDX = 6144      # parity-mask elements per batched mask DMA


def _to_bf(a):
    return np.ascontiguousarray(np.asarray(a, np.float32)).astype(BF16)


def _rep(v):
    v = np.asarray(v, np.float32).reshape(1, -1)
    return np.ascontiguousarray(np.repeat(v, P, 0))


def preprocess(x, edge_index, ln_g, ln_b, W1, b1, bn1_g, bn1_b, bn1_m, bn1_v,
               W2, b2, bn2_g, bn2_b, bn2_m, bn2_v, W3, b3, bn3_g, bn3_b, bn3_m,
               bn3_v, fc1_W, fc1_b, lnc_g, lnc_b, fc2_W, fc2_b):
    N, D = x.shape
    E = edge_index.shape[1]
    H1, H2, H3 = W1.shape[1], W2.shape[1], W3.shape[1]
    HC, C = fc1_W.shape[1], fc2_W.shape[1]
    assert N % NC == 0, N
    NPC = N // NC
    NBLK = (NPC + P - 1) // P
    NPAD = NBLK * P
    NTAB = NC * NPAD
    assert NTAB % 4 == 0
    NHALF = NTAB // 2           # nodes per half-table
    NPAIR = NHALF // 2          # pair elements per half-table
    assert NPAIR <= 32768
    KD = D // P

    src = np.asarray(edge_index[0], np.int64)
    dst = np.asarray(edge_index[1], np.int64)
    deg = np.bincount(dst, minlength=N).astype(np.float32) + 1.0
    dis = 1.0 / np.sqrt(deg)

    # fold LN gain + BN(eval) into weights (identical to v2)
    k1 = bn1_g / np.sqrt(bn1_v + EPS)
    W1f = (np.asarray(ln_g)[:, None] * np.asarray(W1)) * k1[None, :]
    zb1 = (np.asarray(ln_b) @ np.asarray(W1)) * k1
    b1f = np.asarray(b1) * k1 + (bn1_b - bn1_m * k1)
    w1s = W1f.sum(0)
    k2 = bn2_g / np.sqrt(bn2_v + EPS)
    W2f = np.asarray(W2) * k2[None, :]
    b2f = np.asarray(b2) * k2 + (bn2_b - bn2_m * k2)
    k3 = bn3_g / np.sqrt(bn3_v + EPS)
    W3f = np.asarray(W3) * k3[None, :]
    b3f = np.asarray(b3) * k3 + (bn3_b - bn3_m * k3)

    # edges assigned to dst-owner core; srcpad = linear global padded node id
    core_of = dst // NPC
    dloc = dst - core_of * NPC
    srcpad = (src // NPC) * NPAD + (src % NPC)
    half_of = (srcpad // NHALF).astype(np.int64)

    # per (half, block) tile counts, shared across cores
    counts = np.zeros((NC, 2, NBLK), np.int64)
    per_core = []
    for c in range(NC):
        m = core_of == c
        s = srcpad[m]
        d_ = dloc[m]
        h = half_of[m]
        cell = h * NBLK + (d_ >> 7)
        o = np.argsort(cell, kind="stable")
        s, d_, cell = s[o], d_[o], cell[o]
        counts[c] = np.bincount(cell, minlength=2 * NBLK).reshape(2, NBLK)
        per_core.append((s, d_, cell))

    T = -(-counts.max(0) // P)               # [2, NBLK] tiles per cell
    tile_off = np.concatenate([[0], np.cumsum(T.ravel())]).astype(np.int64)
    ntiles = int(tile_off[-1])

    idx16_list, dstrel_list, bmask_list = [], [], []
    for c in range(NC):
        s, d_, cell = per_core[c]
        start = np.searchsorted(cell, np.arange(2 * NBLK))
        pos = np.arange(len(cell)) - start[cell]
        slot = tile_off[cell] * P + pos
        idx_lin = np.zeros(ntiles * P, np.int32)          # pad -> pair 0
        rel_lin = np.full(ntiles * P, 999.0, np.float32)  # pad -> no match
        par_lin = np.zeros(ntiles * P, np.float32)
        loc = s - (s // NHALF) * NHALF                    # id within half
        idx_lin[slot] = (loc >> 1).astype(np.int32)
        par_lin[slot] = (loc & 1).astype(np.float32)
        rel_lin[slot] = (d_ & 127).astype(np.float32)
        assert idx_lin.min() >= 0 and idx_lin.max() < NPAIR
        idx16 = idx_lin.reshape(ntiles * 8, 16).T.astype(np.int16)
        idx16 = np.tile(idx16, (8, 1))                    # [128, ntiles*8]
        dstrel = rel_lin.reshape(ntiles, P).T             # [128, ntiles]
        bmask = par_lin.reshape(ntiles, P).reshape(ntiles * P)
        bmask = np.tile(bmask.reshape(1, -1), (H1, 1))    # [H1, ntiles*128]
        idx16_list.append(np.ascontiguousarray(idx16))
        dstrel_list.append(np.ascontiguousarray(_to_bf(dstrel)))
        bmask_list.append(
            np.ascontiguousarray(bmask.astype(ml_dtypes.float8_e4m3)))

    # per-half groups of blocks: one ap_gather per group, <= GIDX idxs
    groups = {0: [], 1: []}
    for h in range(2):
        b0 = 0
        off = lambda b: int(tile_off[h * NBLK + b])
        while b0 < NBLK:
            nb = 0
            while (b0 + nb < NBLK
                   and (off(b0 + nb) + T[h, b0 + nb] - off(b0)) * P <= GIDX):
                nb += 1
            nb = max(nb, 1)
            groups[h].append((b0, nb))
            b0 += nb

    # per-core node data: x bf16 node-major + feature-major, dis
    xbf = np.asarray(x, np.float32).astype(BF16)
    xp_list, xt_list, disb_list = [], [], []
    for c in range(NC):
        xp = np.zeros((NPAD, D), BF16)
        xp[:NPC] = xbf[c * NPC:(c + 1) * NPC]
        xpm = xp.reshape(NBLK, P, D).transpose(1, 0, 2).reshape(P, NBLK * D)
        xp_list.append(np.ascontiguousarray(xpm))
        xt = np.zeros((P, KD * NPAD), BF16)
        xf = xp.reshape(NPAD, KD, P)
        xt[:] = np.transpose(xf, (2, 1, 0)).reshape(P, KD * NPAD)
        xt_list.append(np.ascontiguousarray(xt))
        db = np.ones(NPAD, np.float32)
        db[:NPC] = dis[c * NPC:(c + 1) * NPC]
        disb_list.append(np.ascontiguousarray(db.reshape(NBLK, P).T))

    iota = np.tile(np.arange(P, dtype=np.float32), (P, 16))
    ident = np.eye(P, dtype=np.float32)

    consts = dict(
        w1=_to_bf(W1f), w2=_to_bf(W2f), w3=_to_bf(W3f),
        fc1w=_to_bf(np.asarray(fc1_W)), fc2w=_to_bf(np.asarray(fc2_W)),
        w1s=_rep(w1s), zb1=_rep(zb1), b1f=_rep(b1f), b2f=_rep(b2f),
        b3f=_rep(b3f), fc1b=_rep(fc1_b), lncg=_rep(lnc_g), lncb=_rep(lnc_b),
        fc2b=_rep(fc2_b), iota=_to_bf(iota), idn=_to_bf(ident),
    )
    in_maps = []
    for c in range(NC):
        m = dict(consts)
        m.update(xp=xp_list[c], xt=xt_list[c], disb=disb_list[c],
                 idx16=idx16_list[c], dstrel=dstrel_list[c],
                 bmask=bmask_list[c])
        in_maps.append(m)

    cfg = dict(N=N, D=D, E=E, H1=H1, H2=H2, H3=H3, HC=HC, C=C, NPC=NPC,
               NBLK=NBLK, NPAD=NPAD, NTAB=NTAB, NPAIR=NPAIR, ntiles=ntiles,
               T=T.tolist(), tile_off=tile_off.tolist(),
               groups={str(k): v for k, v in groups.items()})
    return cfg, in_maps


def build_nc(cfg):
    skip_gather = int(cfg.get("skip_gather", 0))
    skip_sel = int(cfg.get("skip_sel", 0))
    skip_tiles = int(cfg.get("skip_tiles", 0))
    D, H1, H2, H3 = cfg["D"], cfg["H1"], cfg["H2"], cfg["H3"]
    HC, C = cfg["HC"], cfg["C"]
    NBLK, NPAD, NTAB = cfg["NBLK"], cfg["NPAD"], cfg["NTAB"]
    NPAIR = cfg["NPAIR"]
    ntiles, T, tile_off = cfg["ntiles"], cfg["T"], cfg["tile_off"]
    groups = {int(k): v for k, v in cfg["groups"].items()}
    KD = D // P
    HH = [H1, H2, H3]
    # gather channel counts (pad to multiple of 16)
    CHL = [-(-f // 16) * 16 for f in HH]

    nc = bacc.Bacc("TRN2", target_bir_lowering=False, debug=False,
                   num_devices=NC)
    dt = nc.dram_tensor
    ap_xp = dt("xp", [P, NBLK * D], BF, kind="ExternalInput").ap()
    ap_xt = dt("xt", [P, KD * NPAD], BF, kind="ExternalInput").ap()
    ap_disb = dt("disb", [P, NBLK], F32, kind="ExternalInput").ap()
    ap_idx16 = dt("idx16", [P, ntiles * 8], mybir.dt.int16,
                  kind="ExternalInput").ap()
    ap_dstrel = dt("dstrel", [P, ntiles], BF, kind="ExternalInput").ap()
    ap_bmask = dt("bmask", [H1, ntiles * P], F8, kind="ExternalInput").ap()
    ap_w1 = dt("w1", [D, H1], BF, kind="ExternalInput").ap()
    ap_w2 = dt("w2", [H1, H2], BF, kind="ExternalInput").ap()
    ap_w3 = dt("w3", [H2, H3], BF, kind="ExternalInput").ap()
    ap_fc1w = dt("fc1w", [H3, HC], BF, kind="ExternalInput").ap()
    ap_fc2w = dt("fc2w", [HC, C], BF, kind="ExternalInput").ap()
    reps = {}
    for nm, wd in [("w1s", H1), ("zb1", H1), ("b1f", H1), ("b2f", H2),
                   ("b3f", H3), ("fc1b", HC), ("lncg", HC), ("lncb", HC),
                   ("fc2b", C)]:
        reps[nm] = dt(nm, [P, wd], F32, kind="ExternalInput").ap()
    ap_iota = dt("iota", [P, 16 * P], BF, kind="ExternalInput").ap()
    ap_idn = dt("idn", [P, P], BF, kind="ExternalInput").ap()
    ap_out = dt("out", [NPAD, C], F32, kind="ExternalOutput").ap()

    with tile.TileContext(nc) as tc:
        with (
            tc.tile_pool(name="const", bufs=1) as cp,
            tc.tile_pool(name="stage", bufs=1) as st,
            tc.tile_pool(name="work", bufs=3) as wk,
            tc.tile_pool(name="small", bufs=4) as sm,
            tc.tile_pool(name="psA", bufs=2, space="PSUM") as psA,
            tc.tile_pool(name="psZ", bufs=2, space="PSUM") as psZ,
            tc.tile_pool(name="psT", bufs=3, space="PSUM") as psT,
            tc.tile_pool(name="dram", bufs=1, space="DRAM") as dram,
        ):
            def load_const(ap, shape, dtype):
                t = cp.tile(shape, dtype, tag=f"c{ap.tensor.name}",
                            name=f"c{ap.tensor.name}")
                nc.sync.dma_start(t[:], ap)
                return t

            t_w1 = cp.tile([P, KD * H1], BF, tag="w1")
            nc.sync.dma_start(t_w1[:].rearrange("p (k h) -> p k h", h=H1),
                              ap_w1.rearrange("(k p) h -> p k h", p=P))
            t_w2 = load_const(ap_w2, [H1, H2], BF)
            t_w3 = load_const(ap_w3, [H2, H3], BF)
            t_fc1w = load_const(ap_fc1w, [H3, HC], BF)
            t_fc2w = load_const(ap_fc2w, [HC, C], BF)
            t_rep = {}
            for nm in reps:
                t_rep[nm] = load_const(reps[nm], list(reps[nm].shape), F32)
            t_iota = load_const(ap_iota, [P, 16 * P], BF)
            t_idn = load_const(ap_idn, [P, P], BF)
            t_disb = load_const(ap_disb, [P, NBLK], F32)
            t_eps = cp.tile([P, 1], F32, tag="eps")
            nc.vector.memset(t_eps[:], float(EPS))
            t_idx = cp.tile([P, ntiles * 8], mybir.dt.int16, tag="idx")
            nc.sync.dma_start(t_idx[:], ap_idx16)
            t_drel = cp.tile([P, ntiles], BF, tag="drel")
            nc.sync.dma_start(t_drel[:], ap_dstrel)

            # DRAM z tables: local shard (feature-major) + allgathered full
            z_local = [dram.tile([HH[l], NPAD], BF, tag=f"zloc{l}",
                                 name=f"zloc{l}") for l in range(3)]
            z_full = [dram.tile([NC * HH[l], NPAD], BF, tag=f"zfull{l}",
                                name=f"zfull{l}", addr_space="Shared")
                      for l in range(3)]

            zs_buf = [st.tile([P, NBLK * HH[l]], BF, tag=f"zs{l}",
                              name=f"zs{l}") for l in range(3)]
            zs_v = [zs_buf[l][:].rearrange("p (j h) -> p j h", h=HH[l])
                    for l in range(3)]
            t_zT = st.tile([H1, NPAD], BF, tag="zT", name="zT")
            out_buf = st.tile([P, NBLK * C], F32, tag="outb")
            t_hst = st.tile([P, NBLK * H1], F32, tag="hst", name="hst")
            t_hb = st.tile([P, NBLK * H1], BF, tag="hb", name="hb")
            t_z4 = st.tile([P, NBLK * HC], F32, tag="z4", name="z4")
            t_r4 = st.tile([P, NBLK * HC], BF, tag="r4", name="r4")

            def bcast_node(t, w):
                a = t[:]
                return bass.AP(a.tensor, a.offset, a.ap + [[0, w]])

            def bcast_feat(t, w):
                a = t[:]
                return bass.AP(a.tensor, a.offset,
                               [a.ap[0], [0, NBLK], a.ap[1]])

            def transpose_to(dst_sb, src_ap, fh):
                """PE-transpose src [128, fh] -> psum [fh, 128] -> dst sbuf."""
                tp = psT.tile([P, P], BF, tag="tps")
                nc.tensor.transpose(tp[0:fh, :], src_ap, t_idn[:])
                nc.vector.tensor_copy(dst_sb, tp[0:fh, :])

            # ============ phase A: stats + z1 = LN(x) @ W1f (folded) ========
            with tc.tile_pool(name="xin", bufs=1) as xin:
                t_xp = xin.tile([P, NBLK * D], BF, tag="xp")
                nc.sync.dma_start(t_xp[:], ap_xp)
                t_xt = xin.tile([P, KD * NPAD], BF, tag="xt")
                nc.sync.dma_start(t_xt[:], ap_xt)
                t1 = xin.tile([P, NBLK * H1], BF, tag="t1", name="t1")
                t2 = xin.tile([P, NBLK * H1], BF, tag="t2", name="t2")
                ystage = xin.tile([P, NBLK * H1], BF, tag="ystage")

                ssum = sm.tile([P, NBLK], F32, tag="ssum")
                nc.vector.reduce_sum(
                    ssum[:].rearrange("p (j o) -> p j o", o=1),
                    t_xp[:].rearrange("p (j d) -> p j d", d=D),
                    axis=mybir.AxisListType.X)
                s2 = sm.tile([P, NBLK], F32, tag="s2")
                sqscr = wk.tile([P, D], F32, tag="sqscr")
                for b in range(NBLK):
                    nc.scalar.activation(
                        sqscr[:], t_xp[:, b * D:(b + 1) * D],
                        mybir.ActivationFunctionType.Square,
                        accum_out=s2[:, b:b + 1])
                mu = sm.tile([P, NBLK], F32, tag="mu")
                nc.vector.tensor_scalar_mul(mu[:], ssum[:], 1.0 / D)
                musq = sm.tile([P, NBLK], F32, tag="musq")
                nc.vector.tensor_tensor(musq[:], mu[:], mu[:],
                                        op=mybir.AluOpType.mult)
                var = sm.tile([P, NBLK], F32, tag="var")
                nc.vector.tensor_scalar_mul(var[:], s2[:], 1.0 / D)
                nc.vector.tensor_tensor(var[:], var[:], musq[:],
                                        op=mybir.AluOpType.subtract)
                std = sm.tile([P, NBLK], F32, tag="std")
                nc.scalar.activation(std[:], var[:],
                                     mybir.ActivationFunctionType.Sqrt,
                                     bias=t_eps[:], scale=1.0)
                rstd = sm.tile([P, NBLK], F32, tag="rstd")
                nc.vector.reciprocal(rstd[:], std[:])
                a_sc = sm.tile([P, NBLK], F32, tag="a_sc")
                nc.vector.tensor_tensor(a_sc[:], t_disb[:], rstd[:],
                                        op=mybir.AluOpType.mult)
                m2 = sm.tile([P, NBLK], F32, tag="m2")
                nc.vector.tensor_tensor(m2[:], a_sc[:], mu[:],
                                        op=mybir.AluOpType.mult)
                nc.vector.tensor_scalar_mul(m2[:], m2[:], -1.0)

                for b in range(NBLK):
                    zp = psZ.tile([P, H1], F32, tag="zps")
                    for kc in range(KD):
                        nc.tensor.matmul(
                            zp[:],
                            lhsT=t_xt[:, kc * NPAD + b * P:
                                      kc * NPAD + (b + 1) * P],
                            rhs=t_w1[:, kc * H1:(kc + 1) * H1],
                            start=(kc == 0), stop=(kc == KD - 1))
                    nc.vector.tensor_copy(ystage[:, b * H1:(b + 1) * H1],
                                          zp[:])
                nc.vector.tensor_tensor(
                    t1[:].rearrange("p (j h) -> p j h", h=H1),
                    bcast_node(m2, H1), bcast_feat(t_rep["w1s"], H1),
                    op=mybir.AluOpType.mult)
                nc.vector.tensor_tensor(
                    t2[:].rearrange("p (j h) -> p j h", h=H1),
                    bcast_node(t_disb, H1), bcast_feat(t_rep["zb1"], H1),
                    op=mybir.AluOpType.mult)
                nc.vector.tensor_tensor(t1[:], t1[:], t2[:],
                                        op=mybir.AluOpType.add)
                nc.vector.tensor_tensor(
                    t2[:].rearrange("p (j h) -> p j h", h=H1),
                    ystage[:].rearrange("p (j h) -> p j h", h=H1),
                    bcast_node(a_sc, H1), op=mybir.AluOpType.mult)
                nc.vector.tensor_tensor(
                    zs_v[0],
                    t2[:].rearrange("p (j h) -> p j h", h=H1),
                    t1[:].rearrange("p (j h) -> p j h", h=H1),
                    op=mybir.AluOpType.add)

            # z1T = transpose(zs0) feature-major, write local shard
            def write_zT(l):
                fh = HH[l]
                for b in range(NBLK):
                    transpose_to(t_zT[0:fh, b * P:(b + 1) * P],
                                 zs_buf[l][:, b * fh:(b + 1) * fh], fh)
                nc.sync.dma_start(z_local[l][:], t_zT[0:fh, 0:NPAD])

            write_zT(0)

            with (
                tc.tile_pool(name="tabp", bufs=1) as tb,
                tc.tile_pool(name="gath", bufs=2) as gp,
                tc.tile_pool(name="msk", bufs=2) as mp,
                tc.tile_pool(name="onehot", bufs=1) as op_,
                tc.tile_pool(name="ztile", bufs=4) as zp_,
            ):
                # gather table (half): [CH, NPAIR, 2] bf16, NTAB B/partition
                t_tab = tb.tile([P, NTAB // 2], BF, tag="tab", name="tab")
                def edge_layer(l, t_wnext, postbias):
                    Fh = HH[l]
                    ch = CHL[l]
                    nc.gpsimd.collective_compute(
                        "AllGather", mybir.AluOpType.bypass,
                        replica_groups=[list(range(NC))],
                        ins=[z_local[l][:].opt()],
                        outs=[z_full[l][:].opt()],
                    )
                    hstage = t_hst[:, 0:NBLK * Fh]
                    if ch > Fh:
                        # zero the pad feature rows; table loads overwrite
                        # [0:Fh] afterwards (DVE needs 32-aligned partitions)
                        nc.vector.memset(t_tab[0:ch, :], 0.0)
                    for h in range(2):
                        # load half-table: 4 core shards, feature-major
                        for i in range(4):
                            c = 4 * h + i
                            nc.sync.dma_start(
                                t_tab[0:Fh, i * NPAD:(i + 1) * NPAD],
                                z_full[l][c * Fh:(c + 1) * Fh, :])
                        mk, mk_t0, mk_t1 = None, 0, 0
                        for (b0, nb) in groups[h]:
                            t0 = tile_off[h * NBLK + b0]
                            t1_ = tile_off[h * NBLK + b0 + nb - 1] \
                                + T[h][b0 + nb - 1]
                            gt = t1_ - t0
                            if gt == 0:
                                continue
                            if t1_ > mk_t1:
                                # batch parity-mask loads: one DMA covers
                                # several groups' tiles (fewer descriptors)
                                mk_t0 = t0
                                mk_t1 = min(tile_off[(h + 1) * NBLK],
                                            t0 + MIDX // P)
                                mk = mp.tile([Fh, MIDX], F8, tag="mk")
                                nc.sync.dma_start(
                                    mk[:, 0:(mk_t1 - mk_t0) * P],
                                    ap_bmask[0:Fh, mk_t0 * P:mk_t1 * P])
                            g = gp.tile([ch, GIDX * 2], BF, tag="gbuf")
                            if skip_gather:
                                nc.vector.memset(g[:, 0:1], 0.0)
                            else:
                                nc.gpsimd.ap_gather(
                                    out_ap=g[:, 0:gt * P * 2].rearrange(
                                        "p (n d) -> p n d", d=2),
                                    in_ap=t_tab[0:ch, :].rearrange(
                                        "p (n d) -> p n d", d=2),
                                    idxs_ap=t_idx[0:ch, t0 * 8:t1_ * 8],
                                    channels=ch,
                                    num_elems=NPAIR,
                                    d=2,
                                    num_idxs=gt * P,
                                )
                            mo = (t0 - mk_t0) * P
                            ze = gp.tile([Fh, GIDX], BF, tag="ze")
                            gv = g[0:Fh, 0:gt * P * 2].rearrange(
                                "p (n d) -> p n d", d=2)
                            # ze = g0 + (g1 - g0) * parity
                            if skip_sel:
                                nc.vector.memset(ze[:, 0:1], 0.0)
                            else:
                                nc.vector.tensor_tensor(
                                    ze[:, 0:gt * P], gv[:, :, 1], gv[:, :, 0],
                                    op=mybir.AluOpType.subtract)
                                nc.vector.tensor_tensor(
                                    ze[:, 0:gt * P], ze[:, 0:gt * P],
                                    mk[:, mo:mo + gt * P],
                                    op=mybir.AluOpType.mult)
                                nc.vector.tensor_tensor(
                                    ze[:, 0:gt * P], ze[:, 0:gt * P],
                                    gv[:, :, 0], op=mybir.AluOpType.add)
                            # one-hot tiles for this group
                            oh = op_.tile([P, gt * P], BF, tag="ohS")
                            for s0 in range(0, gt, 16):
                                s1 = min(s0 + 16, gt)
                                dr = t_drel[:, t0 + s0:t0 + s1]
                                dr_b = bass.AP(dr.tensor, dr.offset,
                                               dr.ap + [[0, P]])
                                nc.vector.tensor_tensor(
                                    out=oh[:, s0 * P:s1 * P].rearrange(
                                        "p (t w) -> p t w", w=P),
                                    in0=t_iota[:, 0:(s1 - s0) * P].rearrange(
                                        "p (t w) -> p t w", w=P),
                                    in1=dr_b,
                                    op=mybir.AluOpType.is_equal)
                            for b in range(b0, b0 + nb):
                                nt = T[h][b]
                                if nt == 0:
                                    if h == 0:
                                        nc.vector.memset(
                                            hstage[:, b * Fh:(b + 1) * Fh],
                                            0.0)
                                    continue
                                base = tile_off[h * NBLK + b]
                                agg = psA.tile([P, Fh], F32, tag="agg")
                                if skip_tiles:
                                    nc.vector.memset(agg[:], 0.0)
                                for t in range(nt if not skip_tiles else 0):
                                    gidx = base + t - t0
                                    # ze tile [Fh, 128] -> edge-major [128, Fh]
                                    zt = zp_.tile([P, H1], BF, tag="zt")
                                    tp = psT.tile([P, P], BF, tag="tps")
                                    nc.tensor.transpose(
                                        tp[:, 0:Fh],
                                        ze[:, gidx * P:(gidx + 1) * P],
                                        t_idn[0:Fh, 0:Fh])
                                    nc.vector.tensor_copy(zt[:, 0:Fh],
                                                          tp[:, 0:Fh])
                                    nc.tensor.matmul(
                                        agg[:],
                                        lhsT=oh[:, gidx * P:(gidx + 1) * P],
                                        rhs=zt[:, 0:Fh],
                                        start=(t == 0), stop=(t == nt - 1))
                                dst_sl = hstage[:, b * Fh:(b + 1) * Fh]
                                if h == 0:
                                    nc.vector.tensor_copy(dst_sl, agg[:])
                                else:
                                    nc.vector.tensor_tensor(
                                        dst_sl, dst_sl, agg[:],
                                        op=mybir.AluOpType.add)
                    # epilogue: h = relu(disb*(agg + zs) + bias)
                    nc.vector.tensor_tensor(
                        hstage[:].rearrange("p (j h) -> p j h", h=Fh),
                        hstage[:].rearrange("p (j h) -> p j h", h=Fh),
                        zs_v[l], op=mybir.AluOpType.add)
                    nc.vector.tensor_tensor(
                        hstage[:].rearrange("p (j h) -> p j h", h=Fh),
                        hstage[:].rearrange("p (j h) -> p j h", h=Fh),
                        bcast_node(t_disb, Fh), op=mybir.AluOpType.mult)
                    nc.vector.tensor_tensor(
                        hstage[:].rearrange("p (j h) -> p j h", h=Fh),
                        hstage[:].rearrange("p (j h) -> p j h", h=Fh),
                        bcast_feat(postbias, Fh), op=mybir.AluOpType.add)
                    hb = t_hb[:, 0:NBLK * Fh]
                    nc.scalar.activation(hb[:], hstage[:],
                                         mybir.ActivationFunctionType.Relu)
                    if t_wnext is None:
                        return hb
                    # hstage (t_hst) is dead after relu -> reuse it for znext
                    Fo = HH[l + 1]
                    znext = t_hst[:, 0:NBLK * Fo]
                    for b in range(NBLK):
                        hT = wk.tile([P, P], BF, tag="hT")
                        transpose_to(hT[0:Fh, :], hb[:, b * Fh:(b + 1) * Fh],
                                     Fh)
                        zp2 = psZ.tile([P, Fo], F32, tag="zps")
                        nc.tensor.matmul(zp2[:], lhsT=hT[0:Fh, :],
                                         rhs=t_wnext[:], start=True,
                                         stop=True)
                        nc.vector.tensor_copy(znext[:, b * Fo:(b + 1) * Fo],
                                              zp2[:])
                    nc.vector.tensor_tensor(
                        zs_v[l + 1],
                        znext[:].rearrange("p (j h) -> p j h", h=Fo),
                        bcast_node(t_disb, Fo), op=mybir.AluOpType.mult)
                    write_zT(l + 1)
                    return None

                def classifier(h3):
                    z4 = t_z4
                    for b in range(NBLK):
                        hT = wk.tile([P, P], BF, tag="hT")
                        transpose_to(hT[0:H3, :], h3[:, b * H3:(b + 1) * H3],
                                     H3)
                        zp2 = psZ.tile([P, HC], F32, tag="zps")
                        nc.tensor.matmul(zp2[:], lhsT=hT[0:H3, :],
                                         rhs=t_fc1w[:], start=True, stop=True)
                        nc.vector.tensor_copy(z4[:, b * HC:(b + 1) * HC],
                                              zp2[:])
                    nc.vector.tensor_tensor(
                        z4[:].rearrange("p (j h) -> p j h", h=HC),
                        z4[:].rearrange("p (j h) -> p j h", h=HC),
                        bcast_feat(t_rep["fc1b"], HC), op=mybir.AluOpType.add)
                    ssum = sm.tile([P, NBLK], F32, tag="ssum4")
                    nc.vector.reduce_sum(
                        ssum[:].rearrange("p (j o) -> p j o", o=1),
                        z4[:].rearrange("p (j h) -> p j h", h=HC),
                        axis=mybir.AxisListType.X)
                    mu = sm.tile([P, NBLK], F32, tag="mu4")
                    nc.vector.tensor_scalar_mul(mu[:], ssum[:], 1.0 / HC)
                    zc = wk.tile([P, NBLK * HC], F32, tag="zc")
                    nc.vector.tensor_tensor(
                        zc[:].rearrange("p (j h) -> p j h", h=HC),
                        z4[:].rearrange("p (j h) -> p j h", h=HC),
                        bcast_node(mu, HC), op=mybir.AluOpType.subtract)
                    zsq = wk.tile([P, NBLK * HC], F32, tag="zsq")
                    nc.vector.tensor_tensor(zsq[:], zc[:], zc[:],
                                            op=mybir.AluOpType.mult)
                    var = sm.tile([P, NBLK], F32, tag="var4")
                    nc.vector.reduce_sum(
                        var[:].rearrange("p (j o) -> p j o", o=1),
                        zsq[:].rearrange("p (j h) -> p j h", h=HC),
                        axis=mybir.AxisListType.X)
                    nc.vector.tensor_scalar_mul(var[:], var[:], 1.0 / HC)
                    std = sm.tile([P, NBLK], F32, tag="std4")
                    nc.scalar.activation(std[:], var[:],
                                         mybir.ActivationFunctionType.Sqrt,
                                         bias=t_eps[:], scale=1.0)
                    rstd = sm.tile([P, NBLK], F32, tag="rstd4")
                    nc.vector.reciprocal(rstd[:], std[:])
                    nc.vector.tensor_tensor(
                        zc[:].rearrange("p (j h) -> p j h", h=HC),
                        zc[:].rearrange("p (j h) -> p j h", h=HC),
                        bcast_node(rstd, HC), op=mybir.AluOpType.mult)
                    nc.vector.tensor_tensor(
                        zc[:].rearrange("p (j h) -> p j h", h=HC),
                        zc[:].rearrange("p (j h) -> p j h", h=HC),
                        bcast_feat(t_rep["lncg"], HC), op=mybir.AluOpType.mult)
                    nc.vector.tensor_tensor(
                        zc[:].rearrange("p (j h) -> p j h", h=HC),
                        zc[:].rearrange("p (j h) -> p j h", h=HC),
                        bcast_feat(t_rep["lncb"], HC), op=mybir.AluOpType.add)
                    r4 = t_r4
                    nc.scalar.activation(r4[:], zc[:],
                                         mybir.ActivationFunctionType.Relu)
                    for b in range(NBLK):
                        rT = wk.tile([P, P], BF, tag="rT")
                        transpose_to(rT[0:HC, :], r4[:, b * HC:(b + 1) * HC],
                                     HC)
                        op2 = psZ.tile([P, C], F32, tag="zps")
                        nc.tensor.matmul(op2[:], lhsT=rT[0:HC, :],
                                         rhs=t_fc2w[:], start=True, stop=True)
                        nc.vector.tensor_copy(out_buf[:, b * C:(b + 1) * C],
                                              op2[:])
                    nc.vector.tensor_tensor(
                        out_buf[:].rearrange("p (j c) -> p j c", c=C),
                        out_buf[:].rearrange("p (j c) -> p j c", c=C),
                        bcast_feat(t_rep["fc2b"], C), op=mybir.AluOpType.add)

                edge_layer(0, t_w2, t_rep["b1f"])
                edge_layer(1, t_w3, t_rep["b2f"])
                h3 = edge_layer(2, None, t_rep["b3f"])
                classifier(h3)

            nc.sync.dma_start(
                ap_out.rearrange("(j p) c -> p j c", p=P),
                out_buf[:].rearrange("p (j c) -> p j c", c=C))
    nc.compile()
    return nc


_CACHE = {}


def _get_nc(cfg):
    key = repr(sorted((k, str(v)) for k, v in cfg.items()))
    if key not in _CACHE:
        _CACHE[key] = build_nc(cfg)
    return _CACHE[key]


def kernel(**inputs):
    cfg, in_maps = preprocess(**inputs)
    nc = _get_nc(cfg)
    res = bass_utils.run_bass_kernel_spmd(nc, in_maps, core_ids=list(range(NC)))
    NPC, N, C = cfg["NPC"], cfg["N"], cfg["C"]
    out = np.empty((N, C), np.float32)
    for c in range(NC):
        out[c * NPC:(c + 1) * NPC] = res.results[c]["out"][:NPC]
    return out


# revision 4
# speedup vs baseline: 1.2925x; 1.0325x over previous
"""Distributed GCN (3x GCNConv + MLP) on 8 Trainium2 NeuronCores, v3.

v3 strategy: eliminate per-edge DMA descriptors. The v2 kernel gathered
z[src] rows via SWDGE dma_gather (~640k DMA descriptors per core per run),
which dominates the profiled execution span. v3 instead:

  - keeps the full dis-scaled z-table in SBUF, FEATURE-major
    ([F, node-pairs, 2] bf16), AllGathered per layer (bf16, tight rows);
  - gathers per-edge rows with nc.gpsimd.ap_gather — a GPSIMD *compute*
    instruction (~18ns/idx, zero DMA descriptors, one instruction per
    ~2-4k edges). Index granularity is a node PAIR (d=2 elements of 4B),
    so a DVE select with a host-shipped parity mask picks the right node;
  - PE-transposes each 128-edge tile back to edge-major and reuses the
    v2 one-hot PSUM aggregation (f32 accumulate);
  - z tables are bf16 (v2 used fp8) -> better accuracy;
  - the node table is processed in 2 half-table passes (50KB/partition
    each) to fit SBUF; per-block aggregation: pass 0 copies PSUM to an
    SBUF stage, pass 1 adds into it.
"""
import sys

for _p in ("/opt/trn_rl_repo",):
    if _p not in sys.path:
        sys.path.insert(0, _p)

import numpy as np
import ml_dtypes

import concourse.bass as bass
import concourse.bacc as bacc
import concourse.tile as tile
import concourse.mybir as mybir
from concourse import bass_utils

BF16 = ml_dtypes.bfloat16
F32 = mybir.dt.float32
BF = mybir.dt.bfloat16
F8 = mybir.dt.float8e4
EPS = 1e-5
NC = 8
P = 128
GIDX = 2048      # max gather indices per ap_gather call
MIDX = 6144      # parity-mask elements per batched mask DMA


def _to_bf(a):
    return np.ascontiguousarray(np.asarray(a, np.float32)).astype(BF16)


def _rep(v):
    v = np.asarray(v, np.float32).reshape(1, -1)
    return np.ascontiguousarray(np.repeat(v, P, 0))


def preprocess(x, edge_index, ln_g, ln_b, W1, b1, bn1_g, bn1_b, bn1_m, bn1_v,
               W2, b2, bn2_g, bn2_b, bn2_m, bn2_v, W3, b3, bn3_g, bn3_b, bn3_m,
               bn3_v, fc1_W, fc1_b, lnc_g, lnc_b, fc2_W, fc2_b):
    N, D = x.shape
    E = edge_index.shape[1]
    H1, H2, H3 = W1.shape[1], W2.shape[1], W3.shape[1]
    HC, C = fc1_W.shape[1], fc2_W.shape[1]
    assert N % NC == 0, N
    NPC = N // NC
    NBLK = (NPC + P - 1) // P
    NPAD = NBLK * P
    NTAB = NC * NPAD
    assert NTAB % 4 == 0
    NHALF = NTAB // 2           # nodes per half-table
    NPAIR = NHALF // 2          # pair elements per half-table
    assert NPAIR <= 32768
    KD = D // P

    src = np.asarray(edge_index[0], np.int64)
    dst = np.asarray(edge_index[1], np.int64)
    deg = np.bincount(dst, minlength=N).astype(np.float32) + 1.0
    dis = 1.0 / np.sqrt(deg)

    # fold LN gain + BN(eval) into weights (identical to v2)
    k1 = bn1_g / np.sqrt(bn1_v + EPS)
    W1f = (np.asarray(ln_g)[:, None] * np.asarray(W1)) * k1[None, :]
    zb1 = (np.asarray(ln_b) @ np.asarray(W1)) * k1
    b1f = np.asarray(b1) * k1 + (bn1_b - bn1_m * k1)
    w1s = W1f.sum(0)
    k2 = bn2_g / np.sqrt(bn2_v + EPS)
    W2f = np.asarray(W2) * k2[None, :]
    b2f = np.asarray(b2) * k2 + (bn2_b - bn2_m * k2)
    k3 = bn3_g / np.sqrt(bn3_v + EPS)
    W3f = np.asarray(W3) * k3[None, :]
    b3f = np.asarray(b3) * k3 + (bn3_b - bn3_m * k3)

    # edges assigned to dst-owner core; srcpad = linear global padded node id
    core_of = dst // NPC
    dloc = dst - core_of * NPC
    srcpad = (src // NPC) * NPAD + (src % NPC)
    half_of = (srcpad // NHALF).astype(np.int64)

    # per (half, block) tile counts, shared across cores
    counts = np.zeros((NC, 2, NBLK), np.int64)
    per_core = []
    for c in range(NC):
        m = core_of == c
        s = srcpad[m]
        d_ = dloc[m]
        h = half_of[m]
        cell = h * NBLK + (d_ >> 7)
        o = np.argsort(cell, kind="stable")
        s, d_, cell = s[o], d_[o], cell[o]
        counts[c] = np.bincount(cell, minlength=2 * NBLK).reshape(2, NBLK)
        per_core.append((s, d_, cell))

    T = -(-counts.max(0) // P)               # [2, NBLK] tiles per cell
    tile_off = np.concatenate([[0], np.cumsum(T.ravel())]).astype(np.int64)
    ntiles = int(tile_off[-1])

    idx16_list, dstrel_list, bmask_list = [], [], []
    for c in range(NC):
        s, d_, cell = per_core[c]
        start = np.searchsorted(cell, np.arange(2 * NBLK))
        pos = np.arange(len(cell)) - start[cell]
        slot = tile_off[cell] * P + pos
        idx_lin = np.zeros(ntiles * P, np.int32)          # pad -> pair 0
        rel_lin = np.full(ntiles * P, 999.0, np.float32)  # pad -> no match
        par_lin = np.zeros(ntiles * P, np.float32)
        loc = s - (s // NHALF) * NHALF                    # id within half
        idx_lin[slot] = (loc >> 1).astype(np.int32)
        par_lin[slot] = (loc & 1).astype(np.float32)
        rel_lin[slot] = (d_ & 127).astype(np.float32)
        assert idx_lin.min() >= 0 and idx_lin.max() < NPAIR
        idx16 = idx_lin.reshape(ntiles * 8, 16).T.astype(np.int16)
        idx16 = np.tile(idx16, (8, 1))                    # [128, ntiles*8]
        dstrel = rel_lin.reshape(ntiles, P).T             # [128, ntiles]
        bmask = par_lin.reshape(ntiles, P).reshape(ntiles * P)
        bmask = np.tile(bmask.reshape(1, -1), (H1, 1))    # [H1, ntiles*128]
        idx16_list.append(np.ascontiguousarray(idx16))
        dstrel_list.append(np.ascontiguousarray(_to_bf(dstrel)))
        bmask_list.append(
            np.ascontiguousarray(bmask.astype(ml_dtypes.float8_e4m3)))

    # per-half groups of blocks: one ap_gather per group, <= GIDX idxs
    groups = {0: [], 1: []}
    for h in range(2):
        b0 = 0
        off = lambda b: int(tile_off[h * NBLK + b])
        while b0 < NBLK:
            nb = 0
            while (b0 + nb < NBLK
                   and (off(b0 + nb) + T[h, b0 + nb] - off(b0)) * P <= GIDX):
                nb += 1
            nb = max(nb, 1)
            groups[h].append((b0, nb))
            b0 += nb

    # per-core node data: x bf16 node-major + feature-major, dis
    xbf = np.asarray(x, np.float32).astype(BF16)
    xp_list, xt_list, disb_list = [], [], []
    for c in range(NC):
        xp = np.zeros((NPAD, D), BF16)
        xp[:NPC] = xbf[c * NPC:(c + 1) * NPC]
        xpm = xp.reshape(NBLK, P, D).transpose(1, 0, 2).reshape(P, NBLK * D)
        xp_list.append(np.ascontiguousarray(xpm))
        xt = np.zeros((P, KD * NPAD), BF16)
        xf = xp.reshape(NPAD, KD, P)
        xt[:] = np.transpose(xf, (2, 1, 0)).reshape(P, KD * NPAD)
        xt_list.append(np.ascontiguousarray(xt))
        db = np.ones(NPAD, np.float32)
        db[:NPC] = dis[c * NPC:(c + 1) * NPC]
        disb_list.append(np.ascontiguousarray(db.reshape(NBLK, P).T))

    iota = np.tile(np.arange(P, dtype=np.float32), (P, 16))
    ident = np.eye(P, dtype=np.float32)

    consts = dict(
        w1=_to_bf(W1f), w2=_to_bf(W2f), w3=_to_bf(W3f),
        fc1w=_to_bf(np.asarray(fc1_W)), fc2w=_to_bf(np.asarray(fc2_W)),
        w1s=_rep(w1s), zb1=_rep(zb1), b1f=_rep(b1f), b2f=_rep(b2f),
        b3f=_rep(b3f), fc1b=_rep(fc1_b), lncg=_rep(lnc_g), lncb=_rep(lnc_b),
        fc2b=_rep(fc2_b), iota=_to_bf(iota), idn=_to_bf(ident),
    )
    in_maps = []
    for c in range(NC):
        m = dict(consts)
        m.update(xp=xp_list[c], xt=xt_list[c], disb=disb_list[c],
                 idx16=idx16_list[c], dstrel=dstrel_list[c],
                 bmask=bmask_list[c])
        in_maps.append(m)

    cfg = dict(N=N, D=D, E=E, H1=H1, H2=H2, H3=H3, HC=HC, C=C, NPC=NPC,
               NBLK=NBLK, NPAD=NPAD, NTAB=NTAB, NPAIR=NPAIR, ntiles=ntiles,
               T=T.tolist(), tile_off=tile_off.tolist(),
               groups={str(k): v for k, v in groups.items()})
    return cfg, in_maps


def build_nc(cfg):
    tbatch = int(cfg.get("tbatch", 4))
    skip_gather = int(cfg.get("skip_gather", 0))
    skip_sel = int(cfg.get("skip_sel", 0))
    skip_tiles = int(cfg.get("skip_tiles", 0))
    D, H1, H2, H3 = cfg["D"], cfg["H1"], cfg["H2"], cfg["H3"]
    HC, C = cfg["HC"], cfg["C"]
    NBLK, NPAD, NTAB = cfg["NBLK"], cfg["NPAD"], cfg["NTAB"]
    NPAIR = cfg["NPAIR"]
    ntiles, T, tile_off = cfg["ntiles"], cfg["T"], cfg["tile_off"]
    groups = {int(k): v for k, v in cfg["groups"].items()}
    KD = D // P
    HH = [H1, H2, H3]
    # gather channel counts (pad to multiple of 16)
    CHL = [-(-f // 16) * 16 for f in HH]

    nc = bacc.Bacc("TRN2", target_bir_lowering=False, debug=False,
                   num_devices=NC)
    dt = nc.dram_tensor
    ap_xp = dt("xp", [P, NBLK * D], BF, kind="ExternalInput").ap()
    ap_xt = dt("xt", [P, KD * NPAD], BF, kind="ExternalInput").ap()
    ap_disb = dt("disb", [P, NBLK], F32, kind="ExternalInput").ap()
    ap_idx16 = dt("idx16", [P, ntiles * 8], mybir.dt.int16,
                  kind="ExternalInput").ap()
    ap_dstrel = dt("dstrel", [P, ntiles], BF, kind="ExternalInput").ap()
    ap_bmask = dt("bmask", [H1, ntiles * P], F8, kind="ExternalInput").ap()
    ap_w1 = dt("w1", [D, H1], BF, kind="ExternalInput").ap()
    ap_w2 = dt("w2", [H1, H2], BF, kind="ExternalInput").ap()
    ap_w3 = dt("w3", [H2, H3], BF, kind="ExternalInput").ap()
    ap_fc1w = dt("fc1w", [H3, HC], BF, kind="ExternalInput").ap()
    ap_fc2w = dt("fc2w", [HC, C], BF, kind="ExternalInput").ap()
    reps = {}
    for nm, wd in [("w1s", H1), ("zb1", H1), ("b1f", H1), ("b2f", H2),
                   ("b3f", H3), ("fc1b", HC), ("lncg", HC), ("lncb", HC),
                   ("fc2b", C)]:
        reps[nm] = dt(nm, [P, wd], F32, kind="ExternalInput").ap()
    ap_iota = dt("iota", [P, 16 * P], BF, kind="ExternalInput").ap()
    ap_idn = dt("idn", [P, P], BF, kind="ExternalInput").ap()
    ap_out = dt("out", [NPAD, C], F32, kind="ExternalOutput").ap()

    with tile.TileContext(nc) as tc:
        with (
            tc.tile_pool(name="const", bufs=1) as cp,
            tc.tile_pool(name="stage", bufs=1) as st,
            tc.tile_pool(name="work", bufs=3) as wk,
            tc.tile_pool(name="small", bufs=4) as sm,
            tc.tile_pool(name="psA", bufs=2, space="PSUM") as psA,
            tc.tile_pool(name="psZ", bufs=2, space="PSUM") as psZ,
            tc.tile_pool(name="psT", bufs=3, space="PSUM") as psT,
            tc.tile_pool(name="dram", bufs=1, space="DRAM") as dram,
        ):
            def load_const(ap, shape, dtype):
                t = cp.tile(shape, dtype, tag=f"c{ap.tensor.name}",
                            name=f"c{ap.tensor.name}")
                nc.sync.dma_start(t[:], ap)
                return t

            t_w1 = cp.tile([P, KD * H1], BF, tag="w1")
            nc.sync.dma_start(t_w1[:].rearrange("p (k h) -> p k h", h=H1),
                              ap_w1.rearrange("(k p) h -> p k h", p=P))
            t_w2 = load_const(ap_w2, [H1, H2], BF)
            t_w3 = load_const(ap_w3, [H2, H3], BF)
            t_fc1w = load_const(ap_fc1w, [H3, HC], BF)
            t_fc2w = load_const(ap_fc2w, [HC, C], BF)
            t_rep = {}
            for nm in reps:
                t_rep[nm] = load_const(reps[nm], list(reps[nm].shape), F32)
            t_iota = load_const(ap_iota, [P, 16 * P], BF)
            t_idn = load_const(ap_idn, [P, P], BF)
            t_disb = load_const(ap_disb, [P, NBLK], F32)
            t_eps = cp.tile([P, 1], F32, tag="eps")
            nc.vector.memset(t_eps[:], float(EPS))
            t_idx = cp.tile([P, ntiles * 8], mybir.dt.int16, tag="idx")
            nc.sync.dma_start(t_idx[:], ap_idx16)
            t_drel = cp.tile([P, ntiles], BF, tag="drel")
            nc.sync.dma_start(t_drel[:], ap_dstrel)

            # DRAM z tables: local shard (feature-major) + allgathered full
            z_local = [dram.tile([HH[l], NPAD], BF, tag=f"zloc{l}",
                                 name=f"zloc{l}") for l in range(3)]
            z_full = [dram.tile([NC * HH[l], NPAD], BF, tag=f"zfull{l}",
                                name=f"zfull{l}", addr_space="Shared")
                      for l in range(3)]

            zs_buf = [st.tile([P, NBLK * HH[l]], BF, tag=f"zs{l}",
                              name=f"zs{l}") for l in range(3)]
            zs_v = [zs_buf[l][:].rearrange("p (j h) -> p j h", h=HH[l])
                    for l in range(3)]
            t_zT = st.tile([H1, NPAD], BF, tag="zT", name="zT")
            out_buf = st.tile([P, NBLK * C], F32, tag="outb")
            t_hst = st.tile([P, NBLK * H1], F32, tag="hst", name="hst")
            t_hb = st.tile([P, NBLK * H1], BF, tag="hb", name="hb")
            t_z4 = st.tile([P, NBLK * HC], F32, tag="z4", name="z4")
            t_r4 = st.tile([P, NBLK * HC], BF, tag="r4", name="r4")

            def bcast_node(t, w):
                a = t[:]
                return bass.AP(a.tensor, a.offset, a.ap + [[0, w]])

            def bcast_feat(t, w):
                a = t[:]
                return bass.AP(a.tensor, a.offset,
                               [a.ap[0], [0, NBLK], a.ap[1]])

            def transpose_to(dst_sb, src_ap, fh):
                """PE-transpose src [128, fh] -> psum [fh, 128] -> dst sbuf."""
                tp = psT.tile([P, P], BF, tag="tps")
                nc.tensor.transpose(tp[0:fh, :], src_ap, t_idn[:])
                nc.vector.tensor_copy(dst_sb, tp[0:fh, :])

            # ============ phase A: stats + z1 = LN(x) @ W1f (folded) ========
            with tc.tile_pool(name="xin", bufs=1) as xin:
                t_xp = xin.tile([P, NBLK * D], BF, tag="xp")
                nc.sync.dma_start(t_xp[:], ap_xp)
                t_xt = xin.tile([P, KD * NPAD], BF, tag="xt")
                nc.sync.dma_start(t_xt[:], ap_xt)
                t1 = xin.tile([P, NBLK * H1], BF, tag="t1", name="t1")
                t2 = xin.tile([P, NBLK * H1], BF, tag="t2", name="t2")
                ystage = xin.tile([P, NBLK * H1], BF, tag="ystage")

                ssum = sm.tile([P, NBLK], F32, tag="ssum")
                nc.vector.reduce_sum(
                    ssum[:].rearrange("p (j o) -> p j o", o=1),
                    t_xp[:].rearrange("p (j d) -> p j d", d=D),
                    axis=mybir.AxisListType.X)
                s2 = sm.tile([P, NBLK], F32, tag="s2")
                sqscr = wk.tile([P, D], F32, tag="sqscr")
                for b in range(NBLK):
                    nc.scalar.activation(
                        sqscr[:], t_xp[:, b * D:(b + 1) * D],
                        mybir.ActivationFunctionType.Square,
                        accum_out=s2[:, b:b + 1])
                mu = sm.tile([P, NBLK], F32, tag="mu")
                nc.vector.tensor_scalar_mul(mu[:], ssum[:], 1.0 / D)
                musq = sm.tile([P, NBLK], F32, tag="musq")
                nc.vector.tensor_tensor(musq[:], mu[:], mu[:],
                                        op=mybir.AluOpType.mult)
                var = sm.tile([P, NBLK], F32, tag="var")
                nc.vector.tensor_scalar_mul(var[:], s2[:], 1.0 / D)
                nc.vector.tensor_tensor(var[:], var[:], musq[:],
                                        op=mybir.AluOpType.subtract)
                std = sm.tile([P, NBLK], F32, tag="std")
                nc.scalar.activation(std[:], var[:],
                                     mybir.ActivationFunctionType.Sqrt,
                                     bias=t_eps[:], scale=1.0)
                rstd = sm.tile([P, NBLK], F32, tag="rstd")
                nc.vector.reciprocal(rstd[:], std[:])
                a_sc = sm.tile([P, NBLK], F32, tag="a_sc")
                nc.vector.tensor_tensor(a_sc[:], t_disb[:], rstd[:],
                                        op=mybir.AluOpType.mult)
                m2 = sm.tile([P, NBLK], F32, tag="m2")
                nc.vector.tensor_tensor(m2[:], a_sc[:], mu[:],
                                        op=mybir.AluOpType.mult)
                nc.vector.tensor_scalar_mul(m2[:], m2[:], -1.0)

                for b in range(NBLK):
                    zp = psZ.tile([P, H1], F32, tag="zps")
                    for kc in range(KD):
                        nc.tensor.matmul(
                            zp[:],
                            lhsT=t_xt[:, kc * NPAD + b * P:
                                      kc * NPAD + (b + 1) * P],
                            rhs=t_w1[:, kc * H1:(kc + 1) * H1],
                            start=(kc == 0), stop=(kc == KD - 1))
                    nc.vector.tensor_copy(ystage[:, b * H1:(b + 1) * H1],
                                          zp[:])
                nc.vector.tensor_tensor(
                    t1[:].rearrange("p (j h) -> p j h", h=H1),
                    bcast_node(m2, H1), bcast_feat(t_rep["w1s"], H1),
                    op=mybir.AluOpType.mult)
                nc.vector.tensor_tensor(
                    t2[:].rearrange("p (j h) -> p j h", h=H1),
                    bcast_node(t_disb, H1), bcast_feat(t_rep["zb1"], H1),
                    op=mybir.AluOpType.mult)
                nc.vector.tensor_tensor(t1[:], t1[:], t2[:],
                                        op=mybir.AluOpType.add)
                nc.vector.tensor_tensor(
                    t2[:].rearrange("p (j h) -> p j h", h=H1),
                    ystage[:].rearrange("p (j h) -> p j h", h=H1),
                    bcast_node(a_sc, H1), op=mybir.AluOpType.mult)
                nc.vector.tensor_tensor(
                    zs_v[0],
                    t2[:].rearrange("p (j h) -> p j h", h=H1),
                    t1[:].rearrange("p (j h) -> p j h", h=H1),
                    op=mybir.AluOpType.add)

            # z1T = transpose(zs0) feature-major, write local shard
            def write_zT(l):
                fh = HH[l]
                for b in range(NBLK):
                    transpose_to(t_zT[0:fh, b * P:(b + 1) * P],
                                 zs_buf[l][:, b * fh:(b + 1) * fh], fh)
                nc.sync.dma_start(z_local[l][:], t_zT[0:fh, 0:NPAD])

            write_zT(0)

            with (
                tc.tile_pool(name="tabp", bufs=1) as tb,
                tc.tile_pool(name="gath", bufs=2) as gp,
                tc.tile_pool(name="msk", bufs=2) as mp,
                tc.tile_pool(name="onehot", bufs=1) as op_,
                tc.tile_pool(name="ztile", bufs=4) as zp_,
            ):
                # gather table (half): [CH, NPAIR, 2] bf16, NTAB B/partition
                t_tab = tb.tile([P, NTAB // 2], BF, tag="tab", name="tab")
                def edge_layer(l, t_wnext, postbias):
                    Fh = HH[l]
                    ch = CHL[l]
                    nc.gpsimd.collective_compute(
                        "AllGather", mybir.AluOpType.bypass,
                        replica_groups=[list(range(NC))],
                        ins=[z_local[l][:].opt()],
                        outs=[z_full[l][:].opt()],
                    )
                    hstage = t_hst[:, 0:NBLK * Fh]
                    if ch > Fh:
                        # zero the pad feature rows; table loads overwrite
                        # [0:Fh] afterwards (DVE needs 32-aligned partitions)
                        nc.vector.memset(t_tab[0:ch, :], 0.0)
                    for h in range(2):
                        # load half-table: 4 core shards, feature-major
                        for i in range(4):
                            c = 4 * h + i
                            nc.sync.dma_start(
                                t_tab[0:Fh, i * NPAD:(i + 1) * NPAD],
                                z_full[l][c * Fh:(c + 1) * Fh, :])
                        mk, mk_t0, mk_t1 = None, 0, 0
                        for (b0, nb) in groups[h]:
                            t0 = tile_off[h * NBLK + b0]
                            t1_ = tile_off[h * NBLK + b0 + nb - 1] \
                                + T[h][b0 + nb - 1]
                            gt = t1_ - t0
                            if gt == 0:
                                continue
                            if t1_ > mk_t1:
                                # batch parity-mask loads: one DMA covers
                                # several groups' tiles (fewer descriptors)
                                mk_t0 = t0
                                mk_t1 = min(tile_off[(h + 1) * NBLK],
                                            t0 + MIDX // P)
                                mk = mp.tile([Fh, MIDX], F8, tag="mk")
                                nc.sync.dma_start(
                                    mk[:, 0:(mk_t1 - mk_t0) * P],
                                    ap_bmask[0:Fh, mk_t0 * P:mk_t1 * P])
                            g = gp.tile([ch, GIDX * 2], BF, tag="gbuf")
                            if skip_gather:
                                nc.vector.memset(g[:, 0:1], 0.0)
                            else:
                                nc.gpsimd.ap_gather(
                                    out_ap=g[:, 0:gt * P * 2].rearrange(
                                        "p (n d) -> p n d", d=2),
                                    in_ap=t_tab[0:ch, :].rearrange(
                                        "p (n d) -> p n d", d=2),
                                    idxs_ap=t_idx[0:ch, t0 * 8:t1_ * 8],
                                    channels=ch,
                                    num_elems=NPAIR,
                                    d=2,
                                    num_idxs=gt * P,
                                )
                            mo = (t0 - mk_t0) * P
                            ze = gp.tile([Fh, GIDX], BF, tag="ze")
                            gv = g[0:Fh, 0:gt * P * 2].rearrange(
                                "p (n d) -> p n d", d=2)
                            # ze = g0 + (g1 - g0) * parity
                            if skip_sel:
                                nc.vector.memset(ze[:, 0:1], 0.0)
                            else:
                                nc.vector.tensor_tensor(
                                    ze[:, 0:gt * P], gv[:, :, 1], gv[:, :, 0],
                                    op=mybir.AluOpType.subtract)
                                nc.vector.tensor_tensor(
                                    ze[:, 0:gt * P], ze[:, 0:gt * P],
                                    mk[:, mo:mo + gt * P],
                                    op=mybir.AluOpType.mult)
                                nc.vector.tensor_tensor(
                                    ze[:, 0:gt * P], ze[:, 0:gt * P],
                                    gv[:, :, 0], op=mybir.AluOpType.add)
                            # one-hot tiles for this group
                            oh = op_.tile([P, gt * P], BF, tag="ohS")
                            for s0 in range(0, gt, 16):
                                s1 = min(s0 + 16, gt)
                                dr = t_drel[:, t0 + s0:t0 + s1]
                                dr_b = bass.AP(dr.tensor, dr.offset,
                                               dr.ap + [[0, P]])
                                nc.vector.tensor_tensor(
                                    out=oh[:, s0 * P:s1 * P].rearrange(
                                        "p (t w) -> p t w", w=P),
                                    in0=t_iota[:, 0:(s1 - s0) * P].rearrange(
                                        "p (t w) -> p t w", w=P),
                                    in1=dr_b,
                                    op=mybir.AluOpType.is_equal)
                            for b in range(b0, b0 + nb):
                                nt = T[h][b]
                                if nt == 0:
                                    if h == 0:
                                        nc.vector.memset(
                                            hstage[:, b * Fh:(b + 1) * Fh],
                                            0.0)
                                    continue
                                base = tile_off[h * NBLK + b]
                                agg = psA.tile([P, Fh], F32, tag="agg")
                                if skip_tiles:
                                    nc.vector.memset(agg[:], 0.0)
                                nt_ = nt if not skip_tiles else 0
                                # batch tile-transposes into one PSUM tile,
                                # one wide copy, then accumulate matmuls
                                for q0 in range(0, nt_, tbatch):
                                    q1 = min(q0 + tbatch, nt_)
                                    nq = q1 - q0
                                    zt = zp_.tile([P, 4 * H1], BF, tag="zt")
                                    tp = psT.tile([P, 4 * H1], BF, tag="tps")
                                    for k in range(nq):
                                        gidx = base + q0 + k - t0
                                        nc.tensor.transpose(
                                            tp[:, k * Fh:(k + 1) * Fh],
                                            ze[:, gidx * P:(gidx + 1) * P],
                                            t_idn[0:Fh, 0:Fh])
                                    nc.vector.tensor_copy(zt[:, 0:nq * Fh],
                                                          tp[:, 0:nq * Fh])
                                    for k in range(nq):
                                        t = q0 + k
                                        gidx = base + t - t0
                                        nc.tensor.matmul(
                                            agg[:],
                                            lhsT=oh[:,
                                                    gidx * P:(gidx + 1) * P],
                                            rhs=zt[:, k * Fh:(k + 1) * Fh],
                                            start=(t == 0),
                                            stop=(t == nt_ - 1))
                                dst_sl = hstage[:, b * Fh:(b + 1) * Fh]
                                if h == 0:
                                    nc.vector.tensor_copy(dst_sl, agg[:])
                                else:
                                    nc.vector.tensor_tensor(
                                        dst_sl, dst_sl, agg[:],
                                        op=mybir.AluOpType.add)
                    # epilogue: h = relu(disb*(agg + zs) + bias)
                    nc.vector.tensor_tensor(
                        hstage[:].rearrange("p (j h) -> p j h", h=Fh),
                        hstage[:].rearrange("p (j h) -> p j h", h=Fh),
                        zs_v[l], op=mybir.AluOpType.add)
                    nc.vector.tensor_tensor(
                        hstage[:].rearrange("p (j h) -> p j h", h=Fh),
                        hstage[:].rearrange("p (j h) -> p j h", h=Fh),
                        bcast_node(t_disb, Fh), op=mybir.AluOpType.mult)
                    nc.vector.tensor_tensor(
                        hstage[:].rearrange("p (j h) -> p j h", h=Fh),
                        hstage[:].rearrange("p (j h) -> p j h", h=Fh),
                        bcast_feat(postbias, Fh), op=mybir.AluOpType.add)
                    hb = t_hb[:, 0:NBLK * Fh]
                    nc.scalar.activation(hb[:], hstage[:],
                                         mybir.ActivationFunctionType.Relu)
                    if t_wnext is None:
                        return hb
                    # hstage (t_hst) is dead after relu -> reuse it for znext
                    Fo = HH[l + 1]
                    znext = t_hst[:, 0:NBLK * Fo]
                    for b in range(NBLK):
                        hT = wk.tile([P, P], BF, tag="hT")
                        transpose_to(hT[0:Fh, :], hb[:, b * Fh:(b + 1) * Fh],
                                     Fh)
                        zp2 = psZ.tile([P, Fo], F32, tag="zps")
                        nc.tensor.matmul(zp2[:], lhsT=hT[0:Fh, :],
                                         rhs=t_wnext[:], start=True,
                                         stop=True)
                        nc.vector.tensor_copy(znext[:, b * Fo:(b + 1) * Fo],
                                              zp2[:])
                    nc.vector.tensor_tensor(
                        zs_v[l + 1],
                        znext[:].rearrange("p (j h) -> p j h", h=Fo),
                        bcast_node(t_disb, Fo), op=mybir.AluOpType.mult)
                    write_zT(l + 1)
                    return None

                def classifier(h3):
                    z4 = t_z4
                    for b in range(NBLK):
                        hT = wk.tile([P, P], BF, tag="hT")
                        transpose_to(hT[0:H3, :], h3[:, b * H3:(b + 1) * H3],
                                     H3)
                        zp2 = psZ.tile([P, HC], F32, tag="zps")
                        nc.tensor.matmul(zp2[:], lhsT=hT[0:H3, :],
                                         rhs=t_fc1w[:], start=True, stop=True)
                        nc.vector.tensor_copy(z4[:, b * HC:(b + 1) * HC],
                                              zp2[:])
                    nc.vector.tensor_tensor(
                        z4[:].rearrange("p (j h) -> p j h", h=HC),
                        z4[:].rearrange("p (j h) -> p j h", h=HC),
                        bcast_feat(t_rep["fc1b"], HC), op=mybir.AluOpType.add)
                    ssum = sm.tile([P, NBLK], F32, tag="ssum4")
                    nc.vector.reduce_sum(
                        ssum[:].rearrange("p (j o) -> p j o", o=1),
                        z4[:].rearrange("p (j h) -> p j h", h=HC),
                        axis=mybir.AxisListType.X)
                    mu = sm.tile([P, NBLK], F32, tag="mu4")
                    nc.vector.tensor_scalar_mul(mu[:], ssum[:], 1.0 / HC)
                    zc = wk.tile([P, NBLK * HC], F32, tag="zc")
                    nc.vector.tensor_tensor(
                        zc[:].rearrange("p (j h) -> p j h", h=HC),
                        z4[:].rearrange("p (j h) -> p j h", h=HC),
                        bcast_node(mu, HC), op=mybir.AluOpType.subtract)
                    zsq = wk.tile([P, NBLK * HC], F32, tag="zsq")
                    nc.vector.tensor_tensor(zsq[:], zc[:], zc[:],
                                            op=mybir.AluOpType.mult)
                    var = sm.tile([P, NBLK], F32, tag="var4")
                    nc.vector.reduce_sum(
                        var[:].rearrange("p (j o) -> p j o", o=1),
                        zsq[:].rearrange("p (j h) -> p j h", h=HC),
                        axis=mybir.AxisListType.X)
                    nc.vector.tensor_scalar_mul(var[:], var[:], 1.0 / HC)
                    std = sm.tile([P, NBLK], F32, tag="std4")
                    nc.scalar.activation(std[:], var[:],
                                         mybir.ActivationFunctionType.Sqrt,
                                         bias=t_eps[:], scale=1.0)
                    rstd = sm.tile([P, NBLK], F32, tag="rstd4")
                    nc.vector.reciprocal(rstd[:], std[:])
                    nc.vector.tensor_tensor(
                        zc[:].rearrange("p (j h) -> p j h", h=HC),
                        zc[:].rearrange("p (j h) -> p j h", h=HC),
                        bcast_node(rstd, HC), op=mybir.AluOpType.mult)
                    nc.vector.tensor_tensor(
                        zc[:].rearrange("p (j h) -> p j h", h=HC),
                        zc[:].rearrange("p (j h) -> p j h", h=HC),
                        bcast_feat(t_rep["lncg"], HC), op=mybir.AluOpType.mult)
                    nc.vector.tensor_tensor(
                        zc[:].rearrange("p (j h) -> p j h", h=HC),
                        zc[:].rearrange("p (j h) -> p j h", h=HC),
                        bcast_feat(t_rep["lncb"], HC), op=mybir.AluOpType.add)
                    r4 = t_r4
                    nc.scalar.activation(r4[:], zc[:],
                                         mybir.ActivationFunctionType.Relu)
                    for b in range(NBLK):
                        rT = wk.tile([P, P], BF, tag="rT")
                        transpose_to(rT[0:HC, :], r4[:, b * HC:(b + 1) * HC],
                                     HC)
                        op2 = psZ.tile([P, C], F32, tag="zps")
                        nc.tensor.matmul(op2[:], lhsT=rT[0:HC, :],
                                         rhs=t_fc2w[:], start=True, stop=True)
                        nc.vector.tensor_copy(out_buf[:, b * C:(b + 1) * C],
                                              op2[:])
                    nc.vector.tensor_tensor(
                        out_buf[:].rearrange("p (j c) -> p j c", c=C),
                        out_buf[:].rearrange("p (j c) -> p j c", c=C),
                        bcast_feat(t_rep["fc2b"], C), op=mybir.AluOpType.add)

                edge_layer(0, t_w2, t_rep["b1f"])
                edge_layer(1, t_w3, t_rep["b2f"])
                h3 = edge_layer(2, None, t_rep["b3f"])
                classifier(h3)

            nc.sync.dma_start(
                ap_out.rearrange("(j p) c -> p j c", p=P),
                out_buf[:].rearrange("p (j c) -> p j c", c=C))
    nc.compile()
    return nc


_CACHE = {}


def _get_nc(cfg):
    key = repr(sorted((k, str(v)) for k, v in cfg.items()))
    if key not in _CACHE:
        _CACHE[key] = build_nc(cfg)
    return _CACHE[key]


def kernel(**inputs):
    cfg, in_maps = preprocess(**inputs)
    nc = _get_nc(cfg)
    res = bass_utils.run_bass_kernel_spmd(nc, in_maps, core_ids=list(range(NC)))
    NPC, N, C = cfg["NPC"], cfg["N"], cfg["C"]
    out = np.empty((N, C), np.float32)
    for c in range(NC):
        out[c * NPC:(c + 1) * NPC] = res.results[c]["out"][:NPC]
    return out
